# revision 26
# baseline (speedup 1.0000x reference)
"""Trainium2 Bass kernel for nn_AttentionReweightingFusion.

Contract: kernel(**inputs) takes FULL (unsharded) numpy inputs as produced by
setup_inputs() and returns the FULL [16384, 1024] float32 output.

Strategy (pure data parallel over 8 NeuronCores, weights replicated):
  - 2048 batch rows per core, 4 tiles of 512 rows.
  - Inputs staged to HBM as bf16 (features / weights); quality stays fp32 so
    the data-dependent threshold math (ratio > 0.6 etc.) matches the fp32
    reference exactly; quality is also staged pre-transposed ([11, B]) for
    the tiny gate MLPs. Output is written bf16 and upcast on host. Small
    weights/biases are host-packed into two blobs to cut DMA-issue count.
  - Row-major combine (fin = A*base + B*enh) with per-partition scalars; the
    combined features are transposed by the DMA XBAR transpose engine, so
    the tensor engine runs matmuls only.
  - z1 (dc_w1) in bf16; z2 (dc_w2) in fp8e4m3 DoubleRow (weights x64 on
    host, descaled in the tanh). Attention matmul stays bf16 (fp8 there
    fails 2e-2; verified in simulation).
  - The difficulty rank-1 term of z1 is a K=11 matmul against qualT with a
    host-padded row; the mi-MLP input gather is likewise a host-padded
    [11, 32] weight reading qualT directly.
  - MHA with seq=kv=1 reduces to out_proj(v_proj(x)); wv@wo collapsed
    on-device into W_c (wv transposed via XBAR).
  - comp = (E + Dh*st) * fin with E = 1+d/2, Dh = d/2 broadcast to all
    partitions via K=1 PE matmuls; the (w*q_att) output gates are applied
    by the PSUM->SBUF out-copy's per-partition scale (gates transposed to
    row-major via tiny PE transposes).
  - Engine-queue scheduling: PE order is z1(0), [gate/wc fills], z2(t),
    z1(t+1), att(t) so tile t's tanh->comp latency is hidden by tile t+1's
    z1; output stores ride gpsimd SWDGE so the SP HWDGE ring (loads +
    XBARs) never head-of-line blocks on late stores.
"""

import os

import numpy as np

H = 512
B_FULL = 16384
N_CORES = 8
B_CORE = B_FULL // N_CORES          # 2048
TILE_N = 512                        # batch rows per compute tile
N_TILES = B_CORE // TILE_N          # 4
PC = H // 128                       # feature chunks of 128 (4)
RC_TOT = B_CORE // 128              # row chunks per core (16)

_CACHE: dict = {}

# Exposed for test.py after a profiled run
last_exec_time_ns = None
last_trace_path = None
last_scope_times = None

W2_SCALE = 64.0                     # host pre-scale on dc_w2 for fp8


def _build_program(use_bvo=False, use_xbar=True, use_fp8=True):
    import dataclasses
    from contextlib import ExitStack

    import concourse.bacc as bacc
    import concourse.bass as bass
    import concourse.mybir as mybir
    import concourse.tile as tile
    from concourse.masks import make_identity

    dt = mybir.dt
    f32 = dt.float32
    bf16 = dt.bfloat16
    fp8 = dt.float8e4
    AF = mybir.ActivationFunctionType
    OP = mybir.AluOpType
    PM = mybir.MatmulPerfMode

    nc = bacc.Bacc(num_swdge_queues=4)

    # ---------------- DRAM I/O (per-core shapes) ----------------
    d_img = nc.dram_tensor("image_feat", [B_CORE, H], bf16, kind="ExternalInput")
    d_txt = nc.dram_tensor("text_feat", [B_CORE, H], bf16, kind="ExternalInput")
    d_eimg = nc.dram_tensor("enhanced_image_feat", [B_CORE, H], bf16, kind="ExternalInput")
    d_etxt = nc.dram_tensor("enhanced_text_feat", [B_CORE, H], bf16, kind="ExternalInput")
    d_qual = nc.dram_tensor("quality", [B_CORE, 11], f32, kind="ExternalInput")
    d_qualT = nc.dram_tensor("qualT", [11, B_CORE], bf16, kind="ExternalInput")
    d_miss = nc.dram_tensor("missing_f", [B_CORE], f32, kind="ExternalInput")

    # host-packed small-weight blobs (see kernel() for the column layout)
    d_wblob = nc.dram_tensor("wblob", [64, 132], bf16, kind="ExternalInput")
    d_bblob = nc.dram_tensor("bblob", [64, 5], f32, kind="ExternalInput")
    d_dcb = nc.dram_tensor("dcb", [128, 2 * PC], f32, kind="ExternalInput")

    d_dcw1 = nc.dram_tensor("dcw1", [H, H], fp8 if use_fp8 else bf16,
                            kind="ExternalInput")
    d_dcw1_lastp = nc.dram_tensor("dcw1_lastp", [11, H], bf16, kind="ExternalInput")
    d_dcw2x = nc.dram_tensor("dcw2x", [H, H], fp8 if use_fp8 else bf16,
                             kind="ExternalInput")
    d_wv = nc.dram_tensor("wv", [H, H], bf16, kind="ExternalInput")
    d_wo = nc.dram_tensor("wo", [H, H], bf16, kind="ExternalInput")
    d_bvo = nc.dram_tensor("bvo", [H], f32, kind="ExternalInput")

    d_out = nc.dram_tensor("out", [B_CORE, 2 * H], bf16, kind="ExternalOutput")

    with tile.TileContext(nc) as tc, ExitStack() as ctx:
        singles = ctx.enter_context(tc.tile_pool(name="singles", bufs=1))
        inp = ctx.enter_context(tc.tile_pool(name="inp", bufs=8))
        finp = ctx.enter_context(tc.tile_pool(name="finp", bufs=4))
        fintp = ctx.enter_context(tc.tile_pool(name="fintp", bufs=4))
        ps = ctx.enter_context(tc.tile_pool(name="ps", bufs=8, space="PSUM"))
        g1p = ctx.enter_context(tc.tile_pool(name="g1p", bufs=4))
        stp = ctx.enter_context(tc.tile_pool(name="stp", bufs=4))
        uvp = ctx.enter_context(tc.tile_pool(name="uvp", bufs=3))
        compp = ctx.enter_context(tc.tile_pool(name="compp", bufs=4))
        outp = ctx.enter_context(tc.tile_pool(name="outp", bufs=3))
        tmpp = ctx.enter_context(tc.tile_pool(name="tmpp", bufs=5))
        smallp = ctx.enter_context(tc.tile_pool(name="smallp", bufs=2))
        f8p = ctx.enter_context(tc.tile_pool(name="f8p", bufs=4))

        feats = [d_img, d_txt, d_eimg, d_etxt]

        def emit_loads(t):
            in_sb = []
            for dten in feats:
                it = inp.tile([128, PC, TILE_N], bf16, tag="in", name="it")
                nc.sync.dma_start(
                    out=it,
                    in_=dten[t * TILE_N:(t + 1) * TILE_N, :].rearrange(
                        "(c p) f -> p c f", p=128))
                in_sb.append(it)
            return in_sb

        # -------- SP DMA queue, in critical-path order --------
        qual = singles.tile([128, RC_TOT, 11], f32, tag="qual")
        nc.sync.dma_start(out=qual,
                          in_=d_qual.rearrange("(c p) f -> p c f", p=128))
        mrm = singles.tile([128, RC_TOT], f32, tag="mrm")
        nc.sync.dma_start(out=mrm, in_=d_miss.rearrange("(c p) -> p c", p=128))
        qualT = singles.tile([11, B_CORE], bf16, tag="qualT")
        nc.sync.dma_start(out=qualT, in_=d_qualT[:, :])
        in_sb = emit_loads(0)
        dcw1 = singles.tile([128, PC, H], fp8 if use_fp8 else bf16, tag="dcw1")
        nc.sync.dma_start(out=dcw1, in_=d_dcw1.rearrange("(k p) m -> p k m", p=128))
        dcw1_lastp = singles.tile([11, H], bf16, tag="dcw1_lastp")
        nc.sync.dma_start(out=dcw1_lastp, in_=d_dcw1_lastp[:, :])
        wblob = singles.tile([64, 132], bf16, tag="wblob")
        nc.sync.dma_start(out=wblob, in_=d_wblob[:, :])
        bblob = singles.tile([64, 5], f32, tag="bblob")
        nc.sync.dma_start(out=bblob, in_=d_bblob[:, :])
        dcb = singles.tile([128, 2 * PC], f32, tag="dcb")
        nc.sync.dma_start(out=dcb, in_=d_dcb[:, :])
        dT_row = singles.tile([1, B_CORE], bf16, tag="dT_row")
        nc.sync.dma_start(out=dT_row, in_=d_qualT[10:11, :])
        dcw2x = singles.tile([128, PC, H], fp8 if use_fp8 else bf16, tag="dcw2x")
        nc.sync.dma_start(out=dcw2x, in_=d_dcw2x.rearrange("(k p) m -> p k m", p=128))

        trans_ctx = ExitStack()
        transp = trans_ctx.enter_context(tc.tile_pool(name="transp", bufs=1))
        wvT = transp.tile([128, PC, H], bf16, tag="wvT")
        wo_sb = transp.tile([128, PC, H], bf16, tag="wo_sb")
        nc.sync.dma_start(out=wo_sb, in_=d_wo.rearrange("(k p) m -> p k m", p=128))
        if use_xbar:
            # XBAR out [128,4,512] from DRAM in [512,512]: out[p,j,q]=in[q,j*128+p]
            nc.sync.dma_start_transpose(wvT, d_wv[:, :])
            ident = None
        else:
            ident = singles.tile([128, 128], bf16, tag="ident")
            make_identity(nc, ident)
            wv_sb = transp.tile([128, PC, H], bf16, tag="wv_sb")
            nc.sync.dma_start(out=wv_sb, in_=d_wv.rearrange("(k p) m -> p k m", p=128))
            for c in range(PC):
                for fc in range(PC):
                    pst = ps.tile([128, 128], bf16, tag="ps", name="pst")
                    nc.tensor.transpose(pst, wv_sb[:, c, fc * 128:(fc + 1) * 128], ident)
                    nc.vector.tensor_copy(wvT[:, fc, c * 128:(c + 1) * 128], pst)

        bvo_sb = singles.tile([1, H], bf16, tag="bvo_sb")
        nc.gpsimd.dma_start(out=bvo_sb, in_=d_bvo[:].unsqueeze(0))

        # blob views
        qaw1 = wblob[0:11, 0:64]
        qaw2 = wblob[0:64, 64:96]
        qaw3 = wblob[0:32, 96:97]
        miw1p = wblob[0:11, 97:129]
        wdiff = wblob[0:32, 131:132]       # mi_w2[:,0]-mi_w2[:,1] (host)
        qab1 = bblob[0:64, 0:1]
        qab2 = bblob[0:32, 1:2]
        qab3h = bblob[0:1, 2:3]            # qa_b3/2 (host)
        mib1 = bblob[0:32, 3:4]
        db = bblob[0:1, 4:5]               # (mi_b2[0]-mi_b2[1])/2 (host)
        dcb1 = dcb[:, 0:PC]
        dcb2h = dcb[:, PC:2 * PC]          # dc_b2/2 (host)

        # ---------------- exact fp32 per-row coefficient math ----------------
        def sc(tag):
            return singles.tile([128, RC_TOT], f32, tag=tag, name=tag)

        img_imp = qual[:, :, 6:7].rearrange("p c 1 -> p c")
        text_imp = qual[:, :, 7:8].rearrange("p c 1 -> p c")
        img_auth = qual[:, :, 8:9].rearrange("p c 1 -> p c")
        text_auth = qual[:, :, 9:10].rearrange("p c 1 -> p c")

        e0 = sc("e0"); e1 = sc("e1"); e2 = sc("e2")
        nc.vector.tensor_scalar(e0, mrm, 0.5, None, OP.is_lt)
        nc.vector.tensor_scalar(e1, mrm, 1.0, None, OP.is_equal)
        nc.vector.tensor_scalar(e2, mrm, 1.5, None, OP.is_gt)

        den = sc("den"); ratio = sc("ratio")
        nc.vector.scalar_tensor_tensor(den, img_imp, 1e-8, text_imp, OP.add, OP.add)
        nc.vector.reciprocal(den, den)
        nc.vector.tensor_mul(ratio, img_imp, den)
        ghi = sc("ghi"); glo = sc("glo"); si0 = sc("si0"); st0 = sc("st0")
        nc.vector.tensor_scalar(ghi, ratio, 0.6, None, OP.is_gt)
        nc.vector.tensor_scalar(glo, ratio, 0.4, None, OP.is_lt)
        nc.vector.tensor_sub(si0, ghi, glo)
        nc.vector.tensor_scalar(si0, si0, 0.1, 1.0, OP.mult, OP.add)
        nc.vector.tensor_scalar(st0, si0, -1.0, 2.0, OP.mult, OP.add)

        A_i = sc("A_i"); B_i = sc("B_i"); A_t = sc("A_t"); B_t = sc("B_t")
        t_a = sc("t_a"); t_b = sc("t_b")
        # A_i = e0*si0 + e1 + e2*0.3*img_auth
        nc.vector.scalar_tensor_tensor(t_a, img_auth, 0.3, e2, OP.mult, OP.mult)
        nc.vector.tensor_mul(t_b, si0, e0)
        nc.vector.tensor_add(t_a, t_a, t_b)
        nc.vector.tensor_add(A_i, t_a, e1)
        # B_i = e2*(1-img_auth)*img_imp
        nc.vector.tensor_scalar(t_a, img_auth, -1.0, 1.0, OP.mult, OP.add)
        nc.vector.tensor_mul(t_a, t_a, img_imp)
        nc.vector.tensor_mul(B_i, t_a, e2)
        # A_t = e0*st0 + e1*0.3*text_auth + e2
        nc.vector.scalar_tensor_tensor(t_a, text_auth, 0.3, e1, OP.mult, OP.mult)
        nc.vector.tensor_mul(t_b, st0, e0)
        nc.vector.tensor_add(t_a, t_a, t_b)
        nc.vector.tensor_add(A_t, t_a, e2)
        # B_t = e1*(1-text_auth)*text_imp
        nc.vector.tensor_scalar(t_a, text_auth, -1.0, 1.0, OP.mult, OP.add)
        nc.vector.tensor_mul(t_a, t_a, text_imp)
        nc.vector.tensor_mul(B_t, t_a, e1)

        # ---------------- tiny gate MLPs (transposed space) ----------------
        # Emitted before z1(0) on PE; the chain finishes while tile-0's
        # combine/XBAR are still in flight, so it doesn't gate z1(0).
        hq = singles.tile([1, B_CORE], bf16, tag="hq")   # q_att = .5 + .5*hq
        hw = singles.tile([1, B_CORE], bf16, tag="hw")   # img_w = .5 + .5*hw
        for n in range(N_TILES):
            sl = slice(n * TILE_N, (n + 1) * TILE_N)
            ps1 = ps.tile([64, TILE_N], f32, tag="ps", name="ps1")
            nc.tensor.matmul(ps1, qaw1, qualT[:, sl], start=True, stop=True)
            g1q = smallp.tile([64, TILE_N], bf16, tag="qg1", name="g1q")
            nc.scalar.activation(g1q, ps1, AF.Gelu, bias=qab1)
            ps2 = ps.tile([32, TILE_N], f32, tag="ps", name="ps2")
            nc.tensor.matmul(ps2, qaw2, g1q, start=True, stop=True)
            g2q = smallp.tile([32, TILE_N], bf16, tag="qg2", name="g2q")
            nc.scalar.activation(g2q, ps2, AF.Gelu, bias=qab2)
            ps3 = ps.tile([1, TILE_N], f32, tag="ps", name="ps3")
            nc.tensor.matmul(ps3, qaw3, g2q, start=True, stop=True)
            nc.scalar.activation(hq[:, sl], ps3, AF.Tanh, bias=qab3h, scale=0.5)

            psm1 = ps.tile([32, TILE_N], f32, tag="ps", name="psm1")
            nc.tensor.matmul(psm1, miw1p, qualT[:, sl], start=True, stop=True)
            mg = smallp.tile([32, TILE_N], bf16, tag="mg", name="mg")
            nc.scalar.activation(mg, psm1, AF.Gelu, bias=mib1)
            psm2 = ps.tile([1, TILE_N], f32, tag="ps", name="psm2")
            nc.tensor.matmul(psm2, wdiff, mg, start=True, stop=True)
            nc.scalar.activation(hw[:, sl], psm2, AF.Tanh, bias=db, scale=0.5)

        # difficulty rows: dhb = d/2, e = 1 + d/2  (comp = (e + dhb*st)*fin)
        dhb_row = singles.tile([1, B_CORE], bf16, tag="dhb_row")
        nc.vector.tensor_scalar(dhb_row, dT_row, 0.5, None, OP.mult)
        e_row = singles.tile([1, B_CORE], bf16, tag="e_row")
        nc.vector.tensor_scalar(e_row, dT_row, 0.5, 1.0, OP.mult, OP.add)
        ones_row = singles.tile([1, 128], bf16, tag="ones_row")
        nc.vector.memset(ones_row, 1.0)

        # ---------------- main-loop emit helpers ----------------
        fin_specs = [(0, 2, A_i, B_i), (1, 3, A_t, B_t)]

        def emit_combine(t, in_sb):
            """fin_rm[pi] = [128, 4(c), 512(f)] bf16, rows row-major."""
            fin_rm = []
            for pi, (bfi, efi, Ac, Bc) in enumerate(fin_specs):
                ft = finp.tile([128, PC, H], bf16, tag="fin", name="ft")
                for c in range(PC):
                    g = t * PC + c
                    tmp = tmpp.tile([128, H], bf16, tag="ctmp", name="tmp")
                    nc.vector.tensor_scalar(tmp, in_sb[efi][:, c, :],
                                            Bc[:, g:g + 1], None, OP.mult)
                    nc.vector.scalar_tensor_tensor(ft[:, c, :], in_sb[bfi][:, c, :],
                                                   Ac[:, g:g + 1], tmp,
                                                   OP.mult, OP.add)
                fin_rm.append(ft)
            return fin_rm

        def emit_transpose(fin_rm):
            """finT_blk[pi]: [128, c, fc, 128]; block (c,fc) = fin[c*128+q, fc*128+p].

            XBAR semantics with out [128, 16, 128] and SBUF in [128, 2048]:
            out[p, j, q] = in[q, j*128+p], i.e. j = c*4+fc."""
            finT = []
            for pi in range(2):
                fb = fintp.tile([128, PC, PC, 128], bf16, tag="finT", name="fb")
                if use_xbar:
                    nc.sync.dma_start(
                        out=fb, in_=fin_rm[pi].rearrange("p c f -> p (c f)"),
                        transpose=True)
                else:
                    for c in range(PC):
                        for fc in range(PC):
                            pst = ps.tile([128, 128], bf16, tag="ps", name="pst")
                            nc.tensor.transpose(
                                pst, fin_rm[pi][:, c, fc * 128:(fc + 1) * 128], ident)
                            if (c * PC + fc) % 2 == 0:
                                nc.vector.tensor_copy(fb[:, c, fc, :], pst)
                            else:
                                nc.scalar.activation(fb[:, c, fc, :], pst, AF.Copy)
                finT.append(fb)
            return finT

        def finT_view(fb, fc):
            # [128, 4(c), 128(q)] strided view = feature chunk fc, rows linear
            return fb[:, :, fc, :]

        def emit_f8cast(finT):
            """bf16 finT -> fp8 [128, 4(k), 512(rows linear)] on gpsimd."""
            f8T = []
            for pi in range(2):
                f8 = f8p.tile([128, PC, TILE_N], fp8, tag="f8", name="f8")
                for k in range(PC):
                    nc.gpsimd.tensor_copy(f8[:, k, :], finT_view(finT[pi], k))
                f8T.append(f8)
            return f8T

        def emit_z1_g1(t, finT, f8T):
            tsl = slice(t * TILE_N, (t + 1) * TILE_N)
            g1 = [g1p.tile([128, PC, TILE_N], fp8 if use_fp8 else bf16,
                           tag="g1", name="g1") for _ in range(2)]
            g1_scale = 1.0 / W2_SCALE if use_fp8 else 1.0
            for m in range(PC):
                ms = slice(m * 128, (m + 1) * 128)
                zp = [ps.tile([128, TILE_N], f32, tag="ps", name="z1") for _ in range(2)]
                if use_fp8:
                    for kp in range(2):
                        for pi in range(2):
                            nc.tensor.matmul(zp[pi], dcw1[:, 2 * kp:2 * kp + 2, ms],
                                             f8T[pi][:, 2 * kp:2 * kp + 2, :],
                                             perf_mode=PM.DoubleRow,
                                             start=(kp == 0), stop=False)
                else:
                    for k in range(PC):
                        for pi in range(2):
                            nc.tensor.matmul(zp[pi], dcw1[:, k, ms],
                                             finT_view(finT[pi], k),
                                             start=(k == 0), stop=False)
                for pi in range(2):
                    nc.tensor.matmul(zp[pi], dcw1_lastp[:, ms], qualT[:, tsl],
                                     start=False, stop=True)
                    nc.scalar.activation(g1[pi][:, m, :], zp[pi], AF.Gelu,
                                         bias=dcb1[:, m:m + 1], scale=g1_scale)
            return g1

        def rep0(ap2d, n):
            """[128, F] AP -> [128, n(stride 0), F] broadcast view."""
            return dataclasses.replace(ap2d, ap=[ap2d.ap[0], [0, n], *ap2d.ap[1:]])

        def emit_z2_st_pi(t, g1, pi):
            """z2 + tanh for one feature pair; returns st tile [128, 4, 512]."""
            stt = stp.tile([128, PC, TILE_N], bf16, tag="st", name="st")
            for fc in range(PC):
                ms = slice(fc * 128, (fc + 1) * 128)
                zp = ps.tile([128, TILE_N], f32, tag="ps", name="z2")
                if use_fp8:
                    for kp in range(2):
                        nc.tensor.matmul(zp, dcw2x[:, 2 * kp:2 * kp + 2, ms],
                                         g1[pi][:, 2 * kp:2 * kp + 2, :],
                                         perf_mode=PM.DoubleRow,
                                         start=(kp == 0), stop=(kp == 1))
                    st_scale = 0.5 / W2_SCALE
                else:
                    for k in range(PC):
                        nc.tensor.matmul(zp, dcw2x[:, k, ms], g1[pi][:, k, :],
                                         start=(k == 0), stop=(k == PC - 1))
                    st_scale = 0.5
                nc.scalar.activation(stt[:, fc, :], zp, AF.Tanh,
                                     bias=dcb2h[:, fc:fc + 1], scale=st_scale)
            return stt

        def emit_comp_pi(t, stt, finT, pi):
            tsl = slice(t * TILE_N, (t + 1) * TILE_N)
            # comp = (E + Dh*st) * fin, [128, 4(fc), 512] wide ops
            u = uvp.tile([128, PC, TILE_N], bf16, tag="u", name="u")
            nc.vector.tensor_mul(u, stt, rep0(Dhb[:, tsl], PC))
            n2 = uvp.tile([128, PC, TILE_N], bf16, tag="u", name="n2")
            nc.vector.tensor_add(n2, u, rep0(Eb[:, tsl], PC))
            cpt = compp.tile([128, PC, TILE_N], bf16, tag="comp", name="cpt")
            nc.vector.tensor_mul(
                cpt.rearrange("p f (c q) -> p f c q", q=128),
                n2.rearrange("p f (c q) -> p f c q", q=128),
                finT[pi].rearrange("p c f q -> p f c q"))
            return cpt

        def emit_attention_pi(t, cpt, pi, outs):
            ocol = 1 - pi
            for r in range(PC):
                ap = ps.tile([128, H], f32, tag="ps", name="att")
                for k in range(PC):
                    nc.tensor.matmul(ap, cpt[:, k, r * 128:(r + 1) * 128],
                                     wc[:, k, :], start=(k == 0),
                                     stop=(not use_bvo and k == PC - 1))
                if use_bvo:
                    nc.tensor.matmul(ap, ones_row, bvo_sb, start=False, stop=True)
                # gate (w*q_att) applied per-row via the copy's scale
                g = t * PC + r
                wq_col = wq_rm[:, g:g + 1, pi:pi + 1].rearrange("p a b -> p (a b)")
                if (r + pi) % 2 == 0:
                    nc.scalar.activation(outs[ocol][:, r, :], ap, AF.Copy,
                                         scale=wq_col)
                else:
                    nc.vector.tensor_scalar(outs[ocol][:, r, :], ap,
                                            wq_col, None, OP.mult)

        def emit_out_dma(t, outs):
            if t == N_TILES - 1:
                # tail: per-chunk stores on the (now idle) SP queue so the
                # last copies ship immediately instead of one big late DMA
                for ocol in range(2):
                    for r in range(PC):
                        rs = slice(t * TILE_N + r * 128, t * TILE_N + (r + 1) * 128)
                        nc.sync.dma_start(
                            out=d_out[rs, ocol * H:(ocol + 1) * H],
                            in_=outs[ocol][:, r, :])
                return
            for ocol in range(2):
                # gpsimd SWDGE: keeps late stores off the SP HWDGE ring
                nc.gpsimd.dma_start(
                    out=d_out[t * TILE_N:(t + 1) * TILE_N,
                              ocol * H:(ocol + 1) * H].rearrange(
                        "(r p) f -> p r f", p=128),
                    in_=outs[ocol])

        # ---------------- tile 0 head ----------------
        fin_rm = emit_combine(0, in_sb)
        finT = emit_transpose(fin_rm)
        f8T = emit_f8cast(finT) if use_fp8 else None
        in_sb2 = emit_loads(1)
        g1 = emit_z1_g1(0, finT, f8T)

        # ---------------- PE fill work (needed from att(0)/comp(0) on) -------
        # broadcast Dh/E rows to all partitions via K=1 PE matmuls
        Dhb = singles.tile([128, B_CORE], bf16, tag="Dhb")
        Eb = singles.tile([128, B_CORE], bf16, tag="Eb")
        for bi, (row, dst) in enumerate([(dhb_row, Dhb), (e_row, Eb)]):
            for c4 in range(N_TILES):
                sl = slice(c4 * TILE_N, (c4 + 1) * TILE_N)
                psb = ps.tile([128, TILE_N], f32, tag="ps", name="psb")
                nc.tensor.matmul(psb, ones_row, row[:, sl], start=True, stop=True)
                if (bi + c4) % 2 == 0:
                    nc.scalar.activation(dst[:, sl], psb, AF.Copy)
                else:
                    nc.vector.tensor_copy(dst[:, sl], psb)

        # row-major gates wq_rm[:,:,0]=w_t*q (pi=0: img comp -> text out),
        # wq_rm[:,:,1]=w_i*q, via tiny PE transposes of the hq/hw rows.
        hq_rm = singles.tile([128, RC_TOT, 2], bf16, tag="hq_rm")
        for c in range(RC_TOT):
            cs = slice(c * 128, (c + 1) * 128)
            for col, row in ((0, hq), (1, hw)):
                pst = ps.tile([128, 1], bf16, tag="ps", name="pst")
                nc.tensor.transpose(pst, row[:, cs], ones_row[0:1, 0:1])
                nc.vector.tensor_copy(hq_rm[:, c, col:col + 1], pst)
        wq_rm = singles.tile([128, RC_TOT, 2], f32, tag="wq_rm")
        gtmp = singles.tile([128, RC_TOT], f32, tag="gtmp")
        hqc = hq_rm[:, :, 0:1].rearrange("p c 1 -> p c")
        hwc = hq_rm[:, :, 1:2].rearrange("p c 1 -> p c")
        # wq_t = .25(1+hq)(1-hw); wq_i = .25(1+hq)(1+hw)
        nc.vector.tensor_scalar(gtmp, hwc, -0.25, 0.25, OP.mult, OP.add)
        nc.vector.scalar_tensor_tensor(
            wq_rm[:, :, 0:1].rearrange("p c 1 -> p c"), hqc, 1.0, gtmp,
            OP.add, OP.mult)
        nc.vector.tensor_scalar(gtmp, hwc, 0.25, 0.25, OP.mult, OP.add)
        nc.vector.scalar_tensor_tensor(
            wq_rm[:, :, 1:2].rearrange("p c 1 -> p c"), hqc, 1.0, gtmp,
            OP.add, OP.mult)

        # W_c = wv @ wo
        wc = singles.tile([128, PC, H], bf16, tag="wc")
        for ic in range(PC):
            psw = ps.tile([128, H], f32, tag="ps", name="psw")
            for kc in range(PC):
                nc.tensor.matmul(psw, wvT[:, kc, ic * 128:(ic + 1) * 128],
                                 wo_sb[:, kc, :],
                                 start=(kc == 0), stop=(kc == PC - 1))
            nc.scalar.activation(wc[:, ic, :], psw, AF.Copy)
        trans_ctx.close()

        # ---------------- main loop ----------------
        # PE queue: ... z2(t,p0), z2(t,p1), z1(t+1), att(t,p0), att(t,p1) ...
        for t in range(N_TILES):
            if t + 1 < N_TILES:
                fin2 = emit_combine(t + 1, in_sb2)
            st0_t = emit_z2_st_pi(t, g1, 0)
            st1_t = emit_z2_st_pi(t, g1, 1)
            if t + 1 < N_TILES:
                finT2 = emit_transpose(fin2)
                f8T2 = emit_f8cast(finT2) if use_fp8 else None
                if t + 2 < N_TILES:
                    in_sb2 = emit_loads(t + 2)
                g1 = emit_z1_g1(t + 1, finT2, f8T2)
            else:
                finT2 = None
            outs = [outp.tile([128, PC, H], bf16, tag="out", name="ot")
                    for _ in range(2)]
            cpt0 = emit_comp_pi(t, st0_t, finT, 0)
            emit_attention_pi(t, cpt0, 0, outs)
            cpt1 = emit_comp_pi(t, st1_t, finT, 1)
            emit_attention_pi(t, cpt1, 1, outs)
            emit_out_dma(t, outs)
            finT = finT2

    nc.compile()
    _dedupe_ldweights(nc, mybir)
    return nc


def _dedupe_ldweights(nc, mybir):
    """Drop InstLdweights that reload the exact weights already resident in
    the PE array (no intervening loads). Only sync-free LDWs are removed."""
    removed = 0
    for blk in nc.m.functions[0].blocks:
        insts = list(blk.instructions)
        keep = []
        cur = None
        for i in insts:
            if getattr(i, 'engine', None) != mybir.EngineType.PE:
                keep.append(i)
                continue
            t = type(i).__name__
            if t == 'InstLdweights':
                ap = i.ins[0]
                key = (str(ap.memref), ap.offset, str(ap.ap), str(ap.dtype),
                       bool(getattr(i, 'is_transpose', False)),
                       str(getattr(i, 'perf_mode', None)),
                       str(getattr(i, 'tile_position', None)))
                si = i.sync_info
                has_sync = bool(si and (si.on_wait or si.on_update))
                if key == cur and not has_sync:
                    removed += 1
                    continue
                cur = key
                keep.append(i)
            elif t == 'InstMatmult':
                keep.append(i)
            else:
                cur = None
                keep.append(i)
        if removed:
            blk.instructions = keep
    return removed


def _get_program(use_bvo, use_xbar, use_fp8):
    key = ("nc", use_bvo, use_xbar, use_fp8)
    if key not in _CACHE:
        _CACHE[key] = _build_program(use_bvo, use_xbar, use_fp8)
    return _CACHE[key]


def kernel(**inputs) -> np.ndarray:
    global last_exec_time_ns, last_trace_path, last_scope_times
    import ml_dtypes
    from concourse.bass_utils import run_bass_kernel_spmd

    bf = ml_dtypes.bfloat16
    f8 = ml_dtypes.float8_e4m3

    use_xbar = os.environ.get("KERNEL_XBAR", "1") == "1"
    use_fp8 = os.environ.get("KERNEL_FP8", "1") == "1"
    use_bvo = bool(np.any(np.asarray(inputs["bv"])) or
                   np.any(np.asarray(inputs["bo"])))
    nc = _get_program(use_bvo, use_xbar, use_fp8)

    f = {k: np.ascontiguousarray(np.asarray(v, dtype=np.float32))
         for k, v in inputs.items()}
    missing_f = np.ascontiguousarray(
        np.asarray(inputs["missing_type"]).astype(np.float32))

    # host-staged weight transforms (layout/dtype only, plus the exact
    # bias collapse bvo = bv@wo + bo)
    wblob = np.zeros((64, 132), np.float32)
    wblob[0:11, 0:64] = f["qa_w1"]
    wblob[0:64, 64:96] = f["qa_w2"]
    wblob[0:32, 96:97] = f["qa_w3"]
    wblob[6:10, 97:129] = f["mi_w1"]       # padded mi input gather
    wblob[0:32, 131:132] = (f["mi_w2"][:, 0] - f["mi_w2"][:, 1])[:, None]
    bblob = np.zeros((64, 5), np.float32)
    bblob[0:64, 0] = f["qa_b1"]
    bblob[0:32, 1] = f["qa_b2"]
    bblob[0, 2] = f["qa_b3"][0] * 0.5
    bblob[0:32, 3] = f["mi_b1"]
    bblob[0, 4] = (f["mi_b2"][0] - f["mi_b2"][1]) * 0.5
    dcb = np.concatenate([f["dc_b1"].reshape(PC, 128).T,
                          (f["dc_b2"] * 0.5).reshape(PC, 128).T], axis=1)
    dcw1_lastp = np.zeros((11, H), np.float32)
    dcw1_lastp[10] = f["dc_w1"][H]
    bvo = (f["bv"].astype(np.float64) @ f["wo"].astype(np.float64)
           + f["bo"]).astype(np.float32)
    w2x = np.clip(f["dc_w2"] * W2_SCALE, -240.0, 240.0)

    weights = {
        "wblob": wblob.astype(bf),
        "bblob": np.ascontiguousarray(bblob),
        "dcb": np.ascontiguousarray(dcb),
        "dcw1": (np.clip(f["dc_w1"][:H] * W2_SCALE, -240, 240).astype(f8)
                 if use_fp8 else np.ascontiguousarray(f["dc_w1"][:H]).astype(bf)),
        "dcw1_lastp": ((dcw1_lastp * W2_SCALE).astype(bf)
                       if use_fp8 else dcw1_lastp.astype(bf)),
        "dcw2x": w2x.astype(f8) if use_fp8 else f["dc_w2"].astype(bf),
        "wv": f["wv"].astype(bf), "wo": f["wo"].astype(bf), "bvo": bvo,
    }

    feats_bf = {k: f[k].astype(bf) for k in
                ["image_feat", "text_feat", "enhanced_image_feat",
                 "enhanced_text_feat"]}
    qualT_bf = np.ascontiguousarray(f["quality"].T).astype(bf)

    in_maps = []
    for c in range(N_CORES):
        sl = slice(c * B_CORE, (c + 1) * B_CORE)
        m = {k: np.ascontiguousarray(v[sl]) for k, v in feats_bf.items()}
        m["quality"] = f["quality"][sl]
        m["qualT"] = np.ascontiguousarray(qualT_bf[:, sl])
        m["missing_f"] = missing_f[sl]
        m.update(weights)
        in_maps.append(m)

    trace = os.environ.get("KERNEL_TRACE", "0") == "1"
    res = run_bass_kernel_spmd(nc, in_maps, core_ids=list(range(N_CORES)),
                               trace=trace)
    last_exec_time_ns = res.exec_time_ns
    last_scope_times = res.per_core_scope_times
    if res.instructions_and_trace is not None:
        last_trace_path = res.instructions_and_trace[1]

    out = np.empty((B_FULL, 2 * H), dtype=np.float32)
    for c in range(N_CORES):
        out[c * B_CORE:(c + 1) * B_CORE] = res.results[c]["out"].astype(np.float32)
    return out


# revision 27
# speedup vs baseline: 1.2317x; 1.2317x over previous
"""Trainium2 Bass kernel for nn_AttentionReweightingFusion.

Contract: kernel(**inputs) takes FULL (unsharded) numpy inputs as produced by
setup_inputs() and returns the FULL [16384, 1024] float32 output.

Strategy (pure data parallel over 8 NeuronCores, weights replicated):
  - 2048 batch rows per core, 4 tiles of 512 rows.
  - Inputs staged to HBM as bf16 (features / weights); quality stays fp32 so
    the data-dependent threshold math (ratio > 0.6 etc.) matches the fp32
    reference exactly; quality is also staged pre-transposed ([11, B]) for
    the tiny gate MLPs. Output is written bf16 and upcast on host. Small
    weights/biases are host-packed into two blobs to cut DMA-issue count.
  - Row-major combine (fin = A*base + B*enh) with per-partition scalars; the
    combined features are transposed by the DMA XBAR transpose engine, so
    the tensor engine runs matmuls only.
  - z1 (dc_w1) in bf16; z2 (dc_w2) in fp8e4m3 DoubleRow (weights x64 on
    host, descaled in the tanh). Attention matmul stays bf16 (fp8 there
    fails 2e-2; verified in simulation).
  - The difficulty rank-1 term of z1 is a K=11 matmul against qualT with a
    host-padded row; the mi-MLP input gather is likewise a host-padded
    [11, 32] weight reading qualT directly.
  - MHA with seq=kv=1 reduces to out_proj(v_proj(x)); wv@wo collapsed
    on-device into W_c (wv transposed via XBAR).
  - comp = (E + Dh*st) * fin with E = 1+d/2, Dh = d/2 broadcast to all
    partitions via K=1 PE matmuls; the (w*q_att) output gates are applied
    by the PSUM->SBUF out-copy's per-partition scale (gates transposed to
    row-major via tiny PE transposes).
  - Engine-queue scheduling: PE order is z1(0), [gate/wc fills], z2(t),
    z1(t+1), att(t) so tile t's tanh->comp latency is hidden by tile t+1's
    z1; output stores ride gpsimd SWDGE so the SP HWDGE ring (loads +
    XBARs) never head-of-line blocks on late stores.
"""

import os

import numpy as np

H = 512
B_FULL = 16384
N_CORES = 8
B_CORE = B_FULL // N_CORES          # 2048
TILE_N = 512                        # batch rows per compute tile
N_TILES = B_CORE // TILE_N          # 4
PC = H // 128                       # feature chunks of 128 (4)
RC_TOT = B_CORE // 128              # row chunks per core (16)

_CACHE: dict = {}

# Exposed for test.py after a profiled run
last_exec_time_ns = None
last_trace_path = None
last_scope_times = None

W2_SCALE = 64.0                     # host pre-scale on dc_w2 for fp8


def _build_program(use_bvo=False, use_xbar=True, use_fp8=True):
    import dataclasses
    from contextlib import ExitStack

    import concourse.bacc as bacc
    import concourse.bass as bass
    import concourse.mybir as mybir
    import concourse.tile as tile
    from concourse.masks import make_identity

    dt = mybir.dt
    f32 = dt.float32
    bf16 = dt.bfloat16
    fp8 = dt.float8e4
    AF = mybir.ActivationFunctionType
    OP = mybir.AluOpType
    PM = mybir.MatmulPerfMode

    nc = bacc.Bacc(num_swdge_queues=4)

    # ---------------- DRAM I/O (per-core shapes) ----------------
    d_img = nc.dram_tensor("image_feat", [B_CORE, H], bf16, kind="ExternalInput")
    d_txt = nc.dram_tensor("text_feat", [B_CORE, H], bf16, kind="ExternalInput")
    d_eimg = nc.dram_tensor("enhanced_image_feat", [B_CORE, H], bf16, kind="ExternalInput")
    d_etxt = nc.dram_tensor("enhanced_text_feat", [B_CORE, H], bf16, kind="ExternalInput")
    d_qual = nc.dram_tensor("quality", [B_CORE, 11], f32, kind="ExternalInput")
    d_qualT = nc.dram_tensor("qualT", [11, B_CORE], bf16, kind="ExternalInput")
    d_miss = nc.dram_tensor("missing_f", [B_CORE], f32, kind="ExternalInput")

    # host-packed small-weight blobs (see kernel() for the column layout)
    d_wblob = nc.dram_tensor("wblob", [64, 132], bf16, kind="ExternalInput")
    d_bblob = nc.dram_tensor("bblob", [64, 5], f32, kind="ExternalInput")
    d_dcb = nc.dram_tensor("dcb", [128, 2 * PC], f32, kind="ExternalInput")

    d_dcw1 = nc.dram_tensor("dcw1", [H, H], fp8 if use_fp8 else bf16,
                            kind="ExternalInput")
    d_dcw1_lastp = nc.dram_tensor("dcw1_lastp", [11, H], bf16, kind="ExternalInput")
    d_dcw2x = nc.dram_tensor("dcw2x", [H, H], fp8 if use_fp8 else bf16,
                             kind="ExternalInput")
    d_wv = nc.dram_tensor("wv", [H, H], bf16, kind="ExternalInput")
    d_wo = nc.dram_tensor("wo", [H, H], bf16, kind="ExternalInput")
    d_bvo = nc.dram_tensor("bvo", [H], f32, kind="ExternalInput")

    d_out = nc.dram_tensor("out", [B_CORE, 2 * H], bf16, kind="ExternalOutput")

    with tile.TileContext(nc) as tc, ExitStack() as ctx:
        singles = ctx.enter_context(tc.tile_pool(name="singles", bufs=1))
        inp = ctx.enter_context(tc.tile_pool(name="inp", bufs=8))
        finp = ctx.enter_context(tc.tile_pool(name="finp", bufs=4))
        fintp = ctx.enter_context(tc.tile_pool(name="fintp", bufs=4))
        ps = ctx.enter_context(tc.tile_pool(name="ps", bufs=8, space="PSUM"))
        g1p = ctx.enter_context(tc.tile_pool(name="g1p", bufs=4))
        stp = ctx.enter_context(tc.tile_pool(name="stp", bufs=4))
        uvp = ctx.enter_context(tc.tile_pool(name="uvp", bufs=3))
        compp = ctx.enter_context(tc.tile_pool(name="compp", bufs=4))
        outp = ctx.enter_context(tc.tile_pool(name="outp", bufs=3))
        tmpp = ctx.enter_context(tc.tile_pool(name="tmpp", bufs=5))
        smallp = ctx.enter_context(tc.tile_pool(name="smallp", bufs=2))
        f8p = ctx.enter_context(tc.tile_pool(name="f8p", bufs=4))

        feats = [d_img, d_txt, d_eimg, d_etxt]

        def emit_loads(t):
            in_sb = []
            for dten in feats:
                it = inp.tile([128, PC, TILE_N], bf16, tag="in", name="it")
                nc.sync.dma_start(
                    out=it,
                    in_=dten[t * TILE_N:(t + 1) * TILE_N, :].rearrange(
                        "(c p) f -> p c f", p=128))
                in_sb.append(it)
            return in_sb

        # -------- SP DMA queue, in critical-path order --------
        qual = singles.tile([128, RC_TOT, 11], f32, tag="qual")
        nc.sync.dma_start(out=qual,
                          in_=d_qual.rearrange("(c p) f -> p c f", p=128))
        mrm = singles.tile([128, RC_TOT], f32, tag="mrm")
        nc.sync.dma_start(out=mrm, in_=d_miss.rearrange("(c p) -> p c", p=128))
        qualT = singles.tile([11, B_CORE], bf16, tag="qualT")
        nc.sync.dma_start(out=qualT, in_=d_qualT[:, :])
        in_sb = emit_loads(0)
        dcw1 = singles.tile([128, PC, H], fp8 if use_fp8 else bf16, tag="dcw1")
        nc.sync.dma_start(out=dcw1, in_=d_dcw1.rearrange("(k p) m -> p k m", p=128))
        dcw1_lastp = singles.tile([11, H], bf16, tag="dcw1_lastp")
        nc.sync.dma_start(out=dcw1_lastp, in_=d_dcw1_lastp[:, :])
        wblob = singles.tile([64, 132], bf16, tag="wblob")
        nc.sync.dma_start(out=wblob, in_=d_wblob[:, :])
        bblob = singles.tile([64, 5], f32, tag="bblob")
        nc.sync.dma_start(out=bblob, in_=d_bblob[:, :])
        dcb = singles.tile([128, 2 * PC], f32, tag="dcb")
        nc.sync.dma_start(out=dcb, in_=d_dcb[:, :])
        dT_row = singles.tile([1, B_CORE], bf16, tag="dT_row")
        nc.sync.dma_start(out=dT_row, in_=d_qualT[10:11, :])
        dcw2x = singles.tile([128, PC, H], fp8 if use_fp8 else bf16, tag="dcw2x")
        nc.sync.dma_start(out=dcw2x, in_=d_dcw2x.rearrange("(k p) m -> p k m", p=128))

        trans_ctx = ExitStack()
        transp = trans_ctx.enter_context(tc.tile_pool(name="transp", bufs=1))
        wvT = transp.tile([128, PC, H], bf16, tag="wvT")
        wo_sb = transp.tile([128, PC, H], bf16, tag="wo_sb")
        nc.sync.dma_start(out=wo_sb, in_=d_wo.rearrange("(k p) m -> p k m", p=128))
        if use_xbar:
            # XBAR out [128,4,512] from DRAM in [512,512]: out[p,j,q]=in[q,j*128+p]
            nc.sync.dma_start_transpose(wvT, d_wv[:, :])
            ident = None
        else:
            ident = singles.tile([128, 128], bf16, tag="ident")
            make_identity(nc, ident)
            wv_sb = transp.tile([128, PC, H], bf16, tag="wv_sb")
            nc.sync.dma_start(out=wv_sb, in_=d_wv.rearrange("(k p) m -> p k m", p=128))
            for c in range(PC):
                for fc in range(PC):
                    pst = ps.tile([128, 128], bf16, tag="ps", name="pst")
                    nc.tensor.transpose(pst, wv_sb[:, c, fc * 128:(fc + 1) * 128], ident)
                    nc.vector.tensor_copy(wvT[:, fc, c * 128:(c + 1) * 128], pst)

        bvo_sb = singles.tile([1, H], bf16, tag="bvo_sb")
        nc.gpsimd.dma_start(out=bvo_sb, in_=d_bvo[:].unsqueeze(0))

        # blob views
        qaw1 = wblob[0:11, 0:64]
        qaw2 = wblob[0:64, 64:96]
        qaw3 = wblob[0:32, 96:97]
        miw1p = wblob[0:11, 97:129]
        wdiff = wblob[0:32, 131:132]       # mi_w2[:,0]-mi_w2[:,1] (host)
        qab1 = bblob[0:64, 0:1]
        qab2 = bblob[0:32, 1:2]
        qab3h = bblob[0:1, 2:3]            # qa_b3/2 (host)
        mib1 = bblob[0:32, 3:4]
        db = bblob[0:1, 4:5]               # (mi_b2[0]-mi_b2[1])/2 (host)
        dcb1 = dcb[:, 0:PC]
        dcb2h = dcb[:, PC:2 * PC]          # dc_b2/2 (host)

        # ---------------- exact fp32 per-row coefficient math ----------------
        def sc(tag):
            return singles.tile([128, RC_TOT], f32, tag=tag, name=tag)

        img_imp = qual[:, :, 6:7].rearrange("p c 1 -> p c")
        text_imp = qual[:, :, 7:8].rearrange("p c 1 -> p c")
        img_auth = qual[:, :, 8:9].rearrange("p c 1 -> p c")
        text_auth = qual[:, :, 9:10].rearrange("p c 1 -> p c")

        e0 = sc("e0"); e1 = sc("e1"); e2 = sc("e2")
        nc.vector.tensor_scalar(e0, mrm, 0.5, None, OP.is_lt)
        nc.vector.tensor_scalar(e1, mrm, 1.0, None, OP.is_equal)
        nc.vector.tensor_scalar(e2, mrm, 1.5, None, OP.is_gt)

        den = sc("den"); ratio = sc("ratio")
        nc.vector.scalar_tensor_tensor(den, img_imp, 1e-8, text_imp, OP.add, OP.add)
        nc.vector.reciprocal(den, den)
        nc.vector.tensor_mul(ratio, img_imp, den)
        ghi = sc("ghi"); glo = sc("glo"); si0 = sc("si0"); st0 = sc("st0")
        nc.vector.tensor_scalar(ghi, ratio, 0.6, None, OP.is_gt)
        nc.vector.tensor_scalar(glo, ratio, 0.4, None, OP.is_lt)
        nc.vector.tensor_sub(si0, ghi, glo)
        nc.vector.tensor_scalar(si0, si0, 0.1, 1.0, OP.mult, OP.add)
        nc.vector.tensor_scalar(st0, si0, -1.0, 2.0, OP.mult, OP.add)

        A_i = sc("A_i"); B_i = sc("B_i"); A_t = sc("A_t"); B_t = sc("B_t")
        t_a = sc("t_a"); t_b = sc("t_b")
        # A_i = e0*si0 + e1 + e2*0.3*img_auth
        nc.vector.scalar_tensor_tensor(t_a, img_auth, 0.3, e2, OP.mult, OP.mult)
        nc.vector.tensor_mul(t_b, si0, e0)
        nc.vector.tensor_add(t_a, t_a, t_b)
        nc.vector.tensor_add(A_i, t_a, e1)
        # B_i = e2*(1-img_auth)*img_imp
        nc.vector.tensor_scalar(t_a, img_auth, -1.0, 1.0, OP.mult, OP.add)
        nc.vector.tensor_mul(t_a, t_a, img_imp)
        nc.vector.tensor_mul(B_i, t_a, e2)
        # A_t = e0*st0 + e1*0.3*text_auth + e2
        nc.vector.scalar_tensor_tensor(t_a, text_auth, 0.3, e1, OP.mult, OP.mult)
        nc.vector.tensor_mul(t_b, st0, e0)
        nc.vector.tensor_add(t_a, t_a, t_b)
        nc.vector.tensor_add(A_t, t_a, e2)
        # B_t = e1*(1-text_auth)*text_imp
        nc.vector.tensor_scalar(t_a, text_auth, -1.0, 1.0, OP.mult, OP.add)
        nc.vector.tensor_mul(t_a, t_a, text_imp)
        nc.vector.tensor_mul(B_t, t_a, e1)

        # ---------------- tiny gate MLPs (transposed space) ----------------
        # Emitted before z1(0) on PE; the chain finishes while tile-0's
        # combine/XBAR are still in flight, so it doesn't gate z1(0).
        hq = singles.tile([1, B_CORE], bf16, tag="hq")   # q_att = .5 + .5*hq
        hw = singles.tile([1, B_CORE], bf16, tag="hw")   # img_w = .5 + .5*hw
        for n in range(N_TILES):
            sl = slice(n * TILE_N, (n + 1) * TILE_N)
            ps1 = ps.tile([64, TILE_N], f32, tag="ps", name="ps1")
            nc.tensor.matmul(ps1, qaw1, qualT[:, sl], start=True, stop=True)
            g1q = smallp.tile([64, TILE_N], bf16, tag="qg1", name="g1q")
            nc.scalar.activation(g1q, ps1, AF.Gelu, bias=qab1)
            ps2 = ps.tile([32, TILE_N], f32, tag="ps", name="ps2")
            nc.tensor.matmul(ps2, qaw2, g1q, start=True, stop=True)
            g2q = smallp.tile([32, TILE_N], bf16, tag="qg2", name="g2q")
            nc.scalar.activation(g2q, ps2, AF.Gelu, bias=qab2)
            ps3 = ps.tile([1, TILE_N], f32, tag="ps", name="ps3")
            nc.tensor.matmul(ps3, qaw3, g2q, start=True, stop=True)
            nc.scalar.activation(hq[:, sl], ps3, AF.Tanh, bias=qab3h, scale=0.5)

            psm1 = ps.tile([32, TILE_N], f32, tag="ps", name="psm1")
            nc.tensor.matmul(psm1, miw1p, qualT[:, sl], start=True, stop=True)
            mg = smallp.tile([32, TILE_N], bf16, tag="mg", name="mg")
            nc.scalar.activation(mg, psm1, AF.Gelu, bias=mib1)
            psm2 = ps.tile([1, TILE_N], f32, tag="ps", name="psm2")
            nc.tensor.matmul(psm2, wdiff, mg, start=True, stop=True)
            nc.scalar.activation(hw[:, sl], psm2, AF.Tanh, bias=db, scale=0.5)

        # difficulty rows: dhb = d/2, e = 1 + d/2  (comp = (e + dhb*st)*fin)
        dhb_row = singles.tile([1, B_CORE], bf16, tag="dhb_row")
        nc.vector.tensor_scalar(dhb_row, dT_row, 0.5, None, OP.mult)
        e_row = singles.tile([1, B_CORE], bf16, tag="e_row")
        nc.vector.tensor_scalar(e_row, dT_row, 0.5, 1.0, OP.mult, OP.add)
        ones_row = singles.tile([1, 128], bf16, tag="ones_row")
        nc.vector.memset(ones_row, 1.0)

        # ---------------- main-loop emit helpers ----------------
        fin_specs = [(0, 2, A_i, B_i), (1, 3, A_t, B_t)]

        def emit_combine(t, in_sb):
            """fin_rm[pi] = [128, 4(c), 512(f)] bf16, rows row-major."""
            fin_rm = []
            for pi, (bfi, efi, Ac, Bc) in enumerate(fin_specs):
                ft = finp.tile([128, PC, H], bf16, tag="fin", name="ft")
                for c in range(PC):
                    g = t * PC + c
                    tmp = tmpp.tile([128, H], bf16, tag="ctmp", name="tmp")
                    nc.vector.tensor_scalar(tmp, in_sb[efi][:, c, :],
                                            Bc[:, g:g + 1], None, OP.mult)
                    nc.vector.scalar_tensor_tensor(ft[:, c, :], in_sb[bfi][:, c, :],
                                                   Ac[:, g:g + 1], tmp,
                                                   OP.mult, OP.add)
                fin_rm.append(ft)
            return fin_rm

        def emit_transpose(fin_rm):
            """finT_blk[pi]: [128, c, fc, 128]; block (c,fc) = fin[c*128+q, fc*128+p].

            XBAR semantics with out [128, 16, 128] and SBUF in [128, 2048]:
            out[p, j, q] = in[q, j*128+p], i.e. j = c*4+fc."""
            finT = []
            for pi in range(2):
                fb = fintp.tile([128, PC, PC, 128], bf16, tag="finT", name="fb")
                if use_xbar:
                    nc.sync.dma_start(
                        out=fb, in_=fin_rm[pi].rearrange("p c f -> p (c f)"),
                        transpose=True)
                else:
                    for c in range(PC):
                        for fc in range(PC):
                            pst = ps.tile([128, 128], bf16, tag="ps", name="pst")
                            nc.tensor.transpose(
                                pst, fin_rm[pi][:, c, fc * 128:(fc + 1) * 128], ident)
                            if (c * PC + fc) % 2 == 0:
                                nc.vector.tensor_copy(fb[:, c, fc, :], pst)
                            else:
                                nc.scalar.activation(fb[:, c, fc, :], pst, AF.Copy)
                finT.append(fb)
            return finT

        def finT_view(fb, fc):
            # [128, 4(c), 128(q)] strided view = feature chunk fc, rows linear
            return fb[:, :, fc, :]

        def emit_f8cast(finT):
            """bf16 finT -> fp8 [128, 4(k), 512(rows linear)], DVE/ACT split."""
            f8T = []
            for pi in range(2):
                f8 = f8p.tile([128, PC, TILE_N], fp8, tag="f8", name="f8")
                for k in range(PC):
                    if (pi + k) % 2 == 0:
                        nc.vector.tensor_copy(f8[:, k, :], finT_view(finT[pi], k))
                    else:
                        nc.scalar.activation(f8[:, k, :], finT_view(finT[pi], k),
                                             AF.Copy)
                f8T.append(f8)
            return f8T

        def emit_z1_g1(t, finT, f8T):
            tsl = slice(t * TILE_N, (t + 1) * TILE_N)
            g1 = [g1p.tile([128, PC, TILE_N], fp8 if use_fp8 else bf16,
                           tag="g1", name="g1") for _ in range(2)]
            g1_scale = 1.0 / W2_SCALE if use_fp8 else 1.0
            for m in range(PC):
                ms = slice(m * 128, (m + 1) * 128)
                zp = [ps.tile([128, TILE_N], f32, tag="ps", name="z1") for _ in range(2)]
                if use_fp8:
                    for kp in range(2):
                        for pi in range(2):
                            nc.tensor.matmul(zp[pi], dcw1[:, 2 * kp:2 * kp + 2, ms],
                                             f8T[pi][:, 2 * kp:2 * kp + 2, :],
                                             perf_mode=PM.DoubleRow,
                                             start=(kp == 0), stop=False)
                else:
                    for k in range(PC):
                        for pi in range(2):
                            nc.tensor.matmul(zp[pi], dcw1[:, k, ms],
                                             finT_view(finT[pi], k),
                                             start=(k == 0), stop=False)
                for pi in range(2):
                    nc.tensor.matmul(zp[pi], dcw1_lastp[:, ms], qualT[:, tsl],
                                     start=False, stop=True)
                    nc.scalar.activation(g1[pi][:, m, :], zp[pi], AF.Gelu,
                                         bias=dcb1[:, m:m + 1], scale=g1_scale)
            return g1

        def rep0(ap2d, n):
            """[128, F] AP -> [128, n(stride 0), F] broadcast view."""
            return dataclasses.replace(ap2d, ap=[ap2d.ap[0], [0, n], *ap2d.ap[1:]])

        def emit_z2_st_pi(t, g1, pi):
            """z2 + tanh for one feature pair; returns st tile [128, 4, 512]."""
            stt = stp.tile([128, PC, TILE_N], bf16, tag="st", name="st")
            for fc in range(PC):
                ms = slice(fc * 128, (fc + 1) * 128)
                zp = ps.tile([128, TILE_N], f32, tag="ps", name="z2")
                if use_fp8:
                    for kp in range(2):
                        nc.tensor.matmul(zp, dcw2x[:, 2 * kp:2 * kp + 2, ms],
                                         g1[pi][:, 2 * kp:2 * kp + 2, :],
                                         perf_mode=PM.DoubleRow,
                                         start=(kp == 0), stop=(kp == 1))
                    st_scale = 0.5 / W2_SCALE
                else:
                    for k in range(PC):
                        nc.tensor.matmul(zp, dcw2x[:, k, ms], g1[pi][:, k, :],
                                         start=(k == 0), stop=(k == PC - 1))
                    st_scale = 0.5
                nc.scalar.activation(stt[:, fc, :], zp, AF.Tanh,
                                     bias=dcb2h[:, fc:fc + 1], scale=st_scale)
            return stt

        def emit_comp_pi(t, stt, finT, pi):
            tsl = slice(t * TILE_N, (t + 1) * TILE_N)
            # comp = (E + Dh*st) * fin, [128, 4(fc), 512] wide ops
            u = uvp.tile([128, PC, TILE_N], bf16, tag="u", name="u")
            nc.vector.tensor_mul(u, stt, rep0(Dhb[:, tsl], PC))
            n2 = uvp.tile([128, PC, TILE_N], bf16, tag="u", name="n2")
            nc.vector.tensor_add(n2, u, rep0(Eb[:, tsl], PC))
            cpt = compp.tile([128, PC, TILE_N], bf16, tag="comp", name="cpt")
            nc.vector.tensor_mul(
                cpt.rearrange("p f (c q) -> p f c q", q=128),
                n2.rearrange("p f (c q) -> p f c q", q=128),
                finT[pi].rearrange("p c f q -> p f c q"))
            return cpt

        def emit_attention_pi(t, cpt, pi, outs):
            ocol = 1 - pi
            for r in range(PC):
                ap = ps.tile([128, H], f32, tag="ps", name="att")
                for k in range(PC):
                    nc.tensor.matmul(ap, cpt[:, k, r * 128:(r + 1) * 128],
                                     wc[:, k, :], start=(k == 0),
                                     stop=(not use_bvo and k == PC - 1))
                if use_bvo:
                    nc.tensor.matmul(ap, ones_row, bvo_sb, start=False, stop=True)
                # gate (w*q_att) applied per-row via the copy's scale
                g = t * PC + r
                wq_col = wq_rm[:, g:g + 1, pi:pi + 1].rearrange("p a b -> p (a b)")
                if (r + pi) % 2 == 0:
                    nc.scalar.activation(outs[ocol][:, r, :], ap, AF.Copy,
                                         scale=wq_col)
                else:
                    nc.vector.tensor_scalar(outs[ocol][:, r, :], ap,
                                            wq_col, None, OP.mult)

        def emit_out_dma(t, outs):
            if t == N_TILES - 1:
                # tail: per-chunk stores on the (now idle) SP queue so the
                # last copies ship immediately instead of one big late DMA
                for ocol in range(2):
                    for r in range(PC):
                        rs = slice(t * TILE_N + r * 128, t * TILE_N + (r + 1) * 128)
                        nc.sync.dma_start(
                            out=d_out[rs, ocol * H:(ocol + 1) * H],
                            in_=outs[ocol][:, r, :])
                return
            for ocol in range(2):
                # gpsimd SWDGE: keeps late stores off the SP HWDGE ring
                nc.gpsimd.dma_start(
                    out=d_out[t * TILE_N:(t + 1) * TILE_N,
                              ocol * H:(ocol + 1) * H].rearrange(
                        "(r p) f -> p r f", p=128),
                    in_=outs[ocol])

        # ---------------- tile 0 head ----------------
        fin_rm = emit_combine(0, in_sb)
        finT = emit_transpose(fin_rm)
        f8T = emit_f8cast(finT) if use_fp8 else None
        in_sb2 = emit_loads(1)
        g1 = emit_z1_g1(0, finT, f8T)

        # ---------------- PE fill work (needed from att(0)/comp(0) on) -------
        # broadcast Dh/E rows to all partitions via K=1 PE matmuls
        Dhb = singles.tile([128, B_CORE], bf16, tag="Dhb")
        Eb = singles.tile([128, B_CORE], bf16, tag="Eb")
        for bi, (row, dst) in enumerate([(dhb_row, Dhb), (e_row, Eb)]):
            for c4 in range(N_TILES):
                sl = slice(c4 * TILE_N, (c4 + 1) * TILE_N)
                psb = ps.tile([128, TILE_N], f32, tag="ps", name="psb")
                nc.tensor.matmul(psb, ones_row, row[:, sl], start=True, stop=True)
                if (bi + c4) % 2 == 0:
                    nc.scalar.activation(dst[:, sl], psb, AF.Copy)
                else:
                    nc.vector.tensor_copy(dst[:, sl], psb)

        # row-major gates wq_rm[:,:,0]=w_t*q (pi=0: img comp -> text out),
        # wq_rm[:,:,1]=w_i*q, via tiny PE transposes of the hq/hw rows.
        hq_rm = singles.tile([128, RC_TOT, 2], bf16, tag="hq_rm")
        for c in range(RC_TOT):
            cs = slice(c * 128, (c + 1) * 128)
            for col, row in ((0, hq), (1, hw)):
                pst = ps.tile([128, 1], bf16, tag="ps", name="pst")
                nc.tensor.transpose(pst, row[:, cs], ones_row[0:1, 0:1])
                nc.vector.tensor_copy(hq_rm[:, c, col:col + 1], pst)
        wq_rm = singles.tile([128, RC_TOT, 2], f32, tag="wq_rm")
        gtmp = singles.tile([128, RC_TOT], f32, tag="gtmp")
        hqc = hq_rm[:, :, 0:1].rearrange("p c 1 -> p c")
        hwc = hq_rm[:, :, 1:2].rearrange("p c 1 -> p c")
        # wq_t = .25(1+hq)(1-hw); wq_i = .25(1+hq)(1+hw)
        nc.vector.tensor_scalar(gtmp, hwc, -0.25, 0.25, OP.mult, OP.add)
        nc.vector.scalar_tensor_tensor(
            wq_rm[:, :, 0:1].rearrange("p c 1 -> p c"), hqc, 1.0, gtmp,
            OP.add, OP.mult)
        nc.vector.tensor_scalar(gtmp, hwc, 0.25, 0.25, OP.mult, OP.add)
        nc.vector.scalar_tensor_tensor(
            wq_rm[:, :, 1:2].rearrange("p c 1 -> p c"), hqc, 1.0, gtmp,
            OP.add, OP.mult)

        # W_c = wv @ wo
        wc = singles.tile([128, PC, H], bf16, tag="wc")
        for ic in range(PC):
            psw = ps.tile([128, H], f32, tag="ps", name="psw")
            for kc in range(PC):
                nc.tensor.matmul(psw, wvT[:, kc, ic * 128:(ic + 1) * 128],
                                 wo_sb[:, kc, :],
                                 start=(kc == 0), stop=(kc == PC - 1))
            nc.scalar.activation(wc[:, ic, :], psw, AF.Copy)
        trans_ctx.close()

        # ---------------- main loop ----------------
        # PE queue: ... z2(t,p0), z2(t,p1), z1(t+1), att(t,p0), att(t,p1) ...
        for t in range(N_TILES):
            if t + 1 < N_TILES:
                fin2 = emit_combine(t + 1, in_sb2)
            st0_t = emit_z2_st_pi(t, g1, 0)
            st1_t = emit_z2_st_pi(t, g1, 1)
            if t + 1 < N_TILES:
                finT2 = emit_transpose(fin2)
                f8T2 = emit_f8cast(finT2) if use_fp8 else None
                if t + 2 < N_TILES:
                    in_sb2 = emit_loads(t + 2)
                g1 = emit_z1_g1(t + 1, finT2, f8T2)
            else:
                finT2 = None
            outs = [outp.tile([128, PC, H], bf16, tag="out", name="ot")
                    for _ in range(2)]
            cpt0 = emit_comp_pi(t, st0_t, finT, 0)
            emit_attention_pi(t, cpt0, 0, outs)
            cpt1 = emit_comp_pi(t, st1_t, finT, 1)
            emit_attention_pi(t, cpt1, 1, outs)
            emit_out_dma(t, outs)
            finT = finT2

    nc.compile()
    _dedupe_ldweights(nc, mybir)
    return nc


def _dedupe_ldweights(nc, mybir):
    """Drop InstLdweights that reload the exact weights already resident in
    the PE array (no intervening loads). Only sync-free LDWs are removed."""
    removed = 0
    for blk in nc.m.functions[0].blocks:
        insts = list(blk.instructions)
        keep = []
        cur = None
        for i in insts:
            if getattr(i, 'engine', None) != mybir.EngineType.PE:
                keep.append(i)
                continue
            t = type(i).__name__
            if t == 'InstLdweights':
                ap = i.ins[0]
                key = (str(ap.memref), ap.offset, str(ap.ap), str(ap.dtype),
                       bool(getattr(i, 'is_transpose', False)),
                       str(getattr(i, 'perf_mode', None)),
                       str(getattr(i, 'tile_position', None)))
                si = i.sync_info
                has_sync = bool(si and (si.on_wait or si.on_update))
                if key == cur and not has_sync:
                    removed += 1
                    continue
                cur = key
                keep.append(i)
            elif t == 'InstMatmult':
                keep.append(i)
            else:
                cur = None
                keep.append(i)
        if removed:
            blk.instructions = keep
    return removed


def _get_program(use_bvo, use_xbar, use_fp8):
    key = ("nc", use_bvo, use_xbar, use_fp8)
    if key not in _CACHE:
        _CACHE[key] = _build_program(use_bvo, use_xbar, use_fp8)
    return _CACHE[key]


def kernel(**inputs) -> np.ndarray:
    global last_exec_time_ns, last_trace_path, last_scope_times
    import ml_dtypes
    from concourse.bass_utils import run_bass_kernel_spmd

    bf = ml_dtypes.bfloat16
    f8 = ml_dtypes.float8_e4m3

    use_xbar = os.environ.get("KERNEL_XBAR", "1") == "1"
    use_fp8 = os.environ.get("KERNEL_FP8", "1") == "1"
    use_bvo = bool(np.any(np.asarray(inputs["bv"])) or
                   np.any(np.asarray(inputs["bo"])))
    nc = _get_program(use_bvo, use_xbar, use_fp8)

    f = {k: np.ascontiguousarray(np.asarray(v, dtype=np.float32))
         for k, v in inputs.items()}
    missing_f = np.ascontiguousarray(
        np.asarray(inputs["missing_type"]).astype(np.float32))

    # host-staged weight transforms (layout/dtype only, plus the exact
    # bias collapse bvo = bv@wo + bo)
    wblob = np.zeros((64, 132), np.float32)
    wblob[0:11, 0:64] = f["qa_w1"]
    wblob[0:64, 64:96] = f["qa_w2"]
    wblob[0:32, 96:97] = f["qa_w3"]
    wblob[6:10, 97:129] = f["mi_w1"]       # padded mi input gather
    wblob[0:32, 131:132] = (f["mi_w2"][:, 0] - f["mi_w2"][:, 1])[:, None]
    bblob = np.zeros((64, 5), np.float32)
    bblob[0:64, 0] = f["qa_b1"]
    bblob[0:32, 1] = f["qa_b2"]
    bblob[0, 2] = f["qa_b3"][0] * 0.5
    bblob[0:32, 3] = f["mi_b1"]
    bblob[0, 4] = (f["mi_b2"][0] - f["mi_b2"][1]) * 0.5
    dcb = np.concatenate([f["dc_b1"].reshape(PC, 128).T,
                          (f["dc_b2"] * 0.5).reshape(PC, 128).T], axis=1)
    dcw1_lastp = np.zeros((11, H), np.float32)
    dcw1_lastp[10] = f["dc_w1"][H]
    bvo = (f["bv"].astype(np.float64) @ f["wo"].astype(np.float64)
           + f["bo"]).astype(np.float32)
    w2x = np.clip(f["dc_w2"] * W2_SCALE, -240.0, 240.0)

    weights = {
        "wblob": wblob.astype(bf),
        "bblob": np.ascontiguousarray(bblob),
        "dcb": np.ascontiguousarray(dcb),
        "dcw1": (np.clip(f["dc_w1"][:H] * W2_SCALE, -240, 240).astype(f8)
                 if use_fp8 else np.ascontiguousarray(f["dc_w1"][:H]).astype(bf)),
        "dcw1_lastp": ((dcw1_lastp * W2_SCALE).astype(bf)
                       if use_fp8 else dcw1_lastp.astype(bf)),
        "dcw2x": w2x.astype(f8) if use_fp8 else f["dc_w2"].astype(bf),
        "wv": f["wv"].astype(bf), "wo": f["wo"].astype(bf), "bvo": bvo,
    }

    feats_bf = {k: f[k].astype(bf) for k in
                ["image_feat", "text_feat", "enhanced_image_feat",
                 "enhanced_text_feat"]}
    qualT_bf = np.ascontiguousarray(f["quality"].T).astype(bf)

    in_maps = []
    for c in range(N_CORES):
        sl = slice(c * B_CORE, (c + 1) * B_CORE)
        m = {k: np.ascontiguousarray(v[sl]) for k, v in feats_bf.items()}
        m["quality"] = f["quality"][sl]
        m["qualT"] = np.ascontiguousarray(qualT_bf[:, sl])
        m["missing_f"] = missing_f[sl]
        m.update(weights)
        in_maps.append(m)

    trace = os.environ.get("KERNEL_TRACE", "0") == "1"
    res = run_bass_kernel_spmd(nc, in_maps, core_ids=list(range(N_CORES)),
                               trace=trace)
    last_exec_time_ns = res.exec_time_ns
    last_scope_times = res.per_core_scope_times
    if res.instructions_and_trace is not None:
        last_trace_path = res.instructions_and_trace[1]

    out = np.empty((B_FULL, 2 * H), dtype=np.float32)
    for c in range(N_CORES):
        out[c * B_CORE:(c + 1) * B_CORE] = res.results[c]["out"].astype(np.float32)
    return out


# revision 28
# speedup vs baseline: 1.2385x; 1.0055x over previous
"""Trainium2 Bass kernel for nn_AttentionReweightingFusion.

Contract: kernel(**inputs) takes FULL (unsharded) numpy inputs as produced by
setup_inputs() and returns the FULL [16384, 1024] float32 output.

Strategy (pure data parallel over 8 NeuronCores, weights replicated):
  - 2048 batch rows per core, 4 tiles of 512 rows.
  - Inputs staged to HBM as bf16 (features / weights); quality stays fp32 so
    the data-dependent threshold math (ratio > 0.6 etc.) matches the fp32
    reference exactly; quality is also staged pre-transposed ([11, B]) for
    the tiny gate MLPs. Output is written bf16 and upcast on host. Small
    weights/biases are host-packed into two blobs to cut DMA-issue count.
  - Row-major combine (fin = A*base + B*enh) with per-partition scalars; the
    combined features are transposed by the DMA XBAR transpose engine, so
    the tensor engine runs matmuls only.
  - z1 (dc_w1) in bf16; z2 (dc_w2) in fp8e4m3 DoubleRow (weights x64 on
    host, descaled in the tanh). Attention matmul stays bf16 (fp8 there
    fails 2e-2; verified in simulation).
  - The difficulty rank-1 term of z1 is a K=11 matmul against qualT with a
    host-padded row; the mi-MLP input gather is likewise a host-padded
    [11, 32] weight reading qualT directly.
  - MHA with seq=kv=1 reduces to out_proj(v_proj(x)); wv@wo collapsed
    on-device into W_c (wv transposed via XBAR).
  - comp = (E + Dh*st) * fin with E = 1+d/2, Dh = d/2 broadcast to all
    partitions via K=1 PE matmuls; the (w*q_att) output gates are applied
    by the PSUM->SBUF out-copy's per-partition scale (gates transposed to
    row-major via tiny PE transposes).
  - Engine-queue scheduling: PE order is z1(0), [gate/wc fills], z2(t),
    z1(t+1), att(t) so tile t's tanh->comp latency is hidden by tile t+1's
    z1; output stores ride gpsimd SWDGE so the SP HWDGE ring (loads +
    XBARs) never head-of-line blocks on late stores.
"""

import os

import numpy as np

H = 512
B_FULL = 16384
N_CORES = 8
B_CORE = B_FULL // N_CORES          # 2048
TILE_N = 512                        # batch rows per compute tile
N_TILES = B_CORE // TILE_N          # 4
PC = H // 128                       # feature chunks of 128 (4)
RC_TOT = B_CORE // 128              # row chunks per core (16)

_CACHE: dict = {}

# Exposed for test.py after a profiled run
last_exec_time_ns = None
last_trace_path = None
last_scope_times = None

W2_SCALE = 64.0                     # host pre-scale on dc_w2 for fp8


def _build_program(use_bvo=False, use_xbar=True, use_fp8=True):
    import dataclasses
    from contextlib import ExitStack

    import concourse.bacc as bacc
    import concourse.bass as bass
    import concourse.mybir as mybir
    import concourse.tile as tile
    from concourse.masks import make_identity

    dt = mybir.dt
    f32 = dt.float32
    bf16 = dt.bfloat16
    fp8 = dt.float8e4
    AF = mybir.ActivationFunctionType
    OP = mybir.AluOpType
    PM = mybir.MatmulPerfMode

    nc = bacc.Bacc(num_swdge_queues=4)

    # ---------------- DRAM I/O (per-core shapes) ----------------
    d_img = nc.dram_tensor("image_feat", [B_CORE, H], bf16, kind="ExternalInput")
    d_txt = nc.dram_tensor("text_feat", [B_CORE, H], bf16, kind="ExternalInput")
    d_eimg = nc.dram_tensor("enhanced_image_feat", [B_CORE, H], bf16, kind="ExternalInput")
    d_etxt = nc.dram_tensor("enhanced_text_feat", [B_CORE, H], bf16, kind="ExternalInput")
    d_qual = nc.dram_tensor("quality", [B_CORE, 11], f32, kind="ExternalInput")
    d_qualT = nc.dram_tensor("qualT", [11, B_CORE], bf16, kind="ExternalInput")
    d_miss = nc.dram_tensor("missing_f", [B_CORE], f32, kind="ExternalInput")

    # host-packed small-weight blobs (see kernel() for the column layout)
    d_wblob = nc.dram_tensor("wblob", [64, 132], bf16, kind="ExternalInput")
    d_bblob = nc.dram_tensor("bblob", [64, 5], f32, kind="ExternalInput")
    d_dcb = nc.dram_tensor("dcb", [128, 2 * PC], f32, kind="ExternalInput")

    d_dcw1 = nc.dram_tensor("dcw1", [H, H], fp8 if use_fp8 else bf16,
                            kind="ExternalInput")
    d_dcw1_lastp = nc.dram_tensor("dcw1_lastp", [11, H], bf16, kind="ExternalInput")
    d_dcw2x = nc.dram_tensor("dcw2x", [H, H], fp8 if use_fp8 else bf16,
                             kind="ExternalInput")
    d_wv = nc.dram_tensor("wv", [H, H], bf16, kind="ExternalInput")
    d_wo = nc.dram_tensor("wo", [H, H], bf16, kind="ExternalInput")
    d_bvo = nc.dram_tensor("bvo", [H], f32, kind="ExternalInput")

    d_out = nc.dram_tensor("out", [B_CORE, 2 * H], bf16, kind="ExternalOutput")

    with tile.TileContext(nc) as tc, ExitStack() as ctx:
        singles = ctx.enter_context(tc.tile_pool(name="singles", bufs=1))
        inp = ctx.enter_context(tc.tile_pool(name="inp", bufs=8))
        finp = ctx.enter_context(tc.tile_pool(name="finp", bufs=4))
        fintp = ctx.enter_context(tc.tile_pool(name="fintp", bufs=4))
        ps = ctx.enter_context(tc.tile_pool(name="ps", bufs=8, space="PSUM"))
        g1p = ctx.enter_context(tc.tile_pool(name="g1p", bufs=4))
        stp = ctx.enter_context(tc.tile_pool(name="stp", bufs=4))
        uvp = ctx.enter_context(tc.tile_pool(name="uvp", bufs=3))
        compp = ctx.enter_context(tc.tile_pool(name="compp", bufs=4))
        outp = ctx.enter_context(tc.tile_pool(name="outp", bufs=3))
        tmpp = ctx.enter_context(tc.tile_pool(name="tmpp", bufs=5))
        smallp = ctx.enter_context(tc.tile_pool(name="smallp", bufs=2))
        f8p = ctx.enter_context(tc.tile_pool(name="f8p", bufs=4))

        feats = [d_img, d_txt, d_eimg, d_etxt]

        def emit_loads(t):
            in_sb = []
            for dten in feats:
                it = inp.tile([128, PC, TILE_N], bf16, tag="in", name="it")
                nc.sync.dma_start(
                    out=it,
                    in_=dten[t * TILE_N:(t + 1) * TILE_N, :].rearrange(
                        "(c p) f -> p c f", p=128))
                in_sb.append(it)
            return in_sb

        # -------- SP DMA queue, in critical-path order --------
        qual = singles.tile([128, RC_TOT, 11], f32, tag="qual")
        nc.sync.dma_start(out=qual,
                          in_=d_qual.rearrange("(c p) f -> p c f", p=128))
        mrm = singles.tile([128, RC_TOT], f32, tag="mrm")
        nc.sync.dma_start(out=mrm, in_=d_miss.rearrange("(c p) -> p c", p=128))
        qualT = singles.tile([11, B_CORE], bf16, tag="qualT")
        nc.sync.dma_start(out=qualT, in_=d_qualT[:, :])
        in_sb = emit_loads(0)
        dcw1 = singles.tile([128, PC, H], fp8 if use_fp8 else bf16, tag="dcw1")
        nc.sync.dma_start(out=dcw1, in_=d_dcw1.rearrange("(k p) m -> p k m", p=128))
        dcw1_lastp = singles.tile([11, H], bf16, tag="dcw1_lastp")
        nc.sync.dma_start(out=dcw1_lastp, in_=d_dcw1_lastp[:, :])
        wblob = singles.tile([64, 132], bf16, tag="wblob")
        nc.sync.dma_start(out=wblob, in_=d_wblob[:, :])
        bblob = singles.tile([64, 5], f32, tag="bblob")
        nc.sync.dma_start(out=bblob, in_=d_bblob[:, :])
        dcb = singles.tile([128, 2 * PC], f32, tag="dcb")
        nc.sync.dma_start(out=dcb, in_=d_dcb[:, :])
        dT_row = singles.tile([1, B_CORE], bf16, tag="dT_row")
        nc.sync.dma_start(out=dT_row, in_=d_qualT[10:11, :])
        dcw2x = singles.tile([128, PC, H], fp8 if use_fp8 else bf16, tag="dcw2x")
        nc.sync.dma_start(out=dcw2x, in_=d_dcw2x.rearrange("(k p) m -> p k m", p=128))

        trans_ctx = ExitStack()
        transp = trans_ctx.enter_context(tc.tile_pool(name="transp", bufs=1))
        wvT = transp.tile([128, PC, H], bf16, tag="wvT")
        wo_sb = transp.tile([128, PC, H], bf16, tag="wo_sb")
        nc.sync.dma_start(out=wo_sb, in_=d_wo.rearrange("(k p) m -> p k m", p=128))
        if use_xbar:
            # XBAR out [128,4,512] from DRAM in [512,512]: out[p,j,q]=in[q,j*128+p]
            nc.sync.dma_start_transpose(wvT, d_wv[:, :])
            ident = None
        else:
            ident = singles.tile([128, 128], bf16, tag="ident")
            make_identity(nc, ident)
            wv_sb = transp.tile([128, PC, H], bf16, tag="wv_sb")
            nc.sync.dma_start(out=wv_sb, in_=d_wv.rearrange("(k p) m -> p k m", p=128))
            for c in range(PC):
                for fc in range(PC):
                    pst = ps.tile([128, 128], bf16, tag="ps", name="pst")
                    nc.tensor.transpose(pst, wv_sb[:, c, fc * 128:(fc + 1) * 128], ident)
                    nc.vector.tensor_copy(wvT[:, fc, c * 128:(c + 1) * 128], pst)

        bvo_sb = singles.tile([1, H], bf16, tag="bvo_sb")
        nc.gpsimd.dma_start(out=bvo_sb, in_=d_bvo[:].unsqueeze(0))

        # blob views
        qaw1 = wblob[0:11, 0:64]
        qaw2 = wblob[0:64, 64:96]
        qaw3 = wblob[0:32, 96:97]
        miw1p = wblob[0:11, 97:129]
        wdiff = wblob[0:32, 131:132]       # mi_w2[:,0]-mi_w2[:,1] (host)
        qab1 = bblob[0:64, 0:1]
        qab2 = bblob[0:32, 1:2]
        qab3h = bblob[0:1, 2:3]            # qa_b3/2 (host)
        mib1 = bblob[0:32, 3:4]
        db = bblob[0:1, 4:5]               # (mi_b2[0]-mi_b2[1])/2 (host)
        dcb1 = dcb[:, 0:PC]
        dcb2h = dcb[:, PC:2 * PC]          # dc_b2/2 (host)

        # ---------------- PE warmup burst ----------------
        # The HAM clock gate holds the PE at 1.2 GHz until ~3.4us of sustained
        # matmul activity. Burn idle prologue time on dummy matmuls so all
        # real matmuls (from the MLPs on) run at 2.4 GHz.
        warm = singles.tile([128, TILE_N], bf16, tag="warm")
        nc.vector.memset(warm, 1.0)
        for i in range(16):
            wps = ps.tile([128, TILE_N], f32, tag="ps", name="wps")
            nc.tensor.matmul(wps, warm[:, 0:128], warm, start=True, stop=True)

        # ---------------- exact fp32 per-row coefficient math ----------------
        def sc(tag):
            return singles.tile([128, RC_TOT], f32, tag=tag, name=tag)

        img_imp = qual[:, :, 6:7].rearrange("p c 1 -> p c")
        text_imp = qual[:, :, 7:8].rearrange("p c 1 -> p c")
        img_auth = qual[:, :, 8:9].rearrange("p c 1 -> p c")
        text_auth = qual[:, :, 9:10].rearrange("p c 1 -> p c")

        e0 = sc("e0"); e1 = sc("e1"); e2 = sc("e2")
        nc.vector.tensor_scalar(e0, mrm, 0.5, None, OP.is_lt)
        nc.vector.tensor_scalar(e1, mrm, 1.0, None, OP.is_equal)
        nc.vector.tensor_scalar(e2, mrm, 1.5, None, OP.is_gt)

        den = sc("den"); ratio = sc("ratio")
        nc.vector.scalar_tensor_tensor(den, img_imp, 1e-8, text_imp, OP.add, OP.add)
        nc.vector.reciprocal(den, den)
        nc.vector.tensor_mul(ratio, img_imp, den)
        ghi = sc("ghi"); glo = sc("glo"); si0 = sc("si0"); st0 = sc("st0")
        nc.vector.tensor_scalar(ghi, ratio, 0.6, None, OP.is_gt)
        nc.vector.tensor_scalar(glo, ratio, 0.4, None, OP.is_lt)
        nc.vector.tensor_sub(si0, ghi, glo)
        nc.vector.tensor_scalar(si0, si0, 0.1, 1.0, OP.mult, OP.add)
        nc.vector.tensor_scalar(st0, si0, -1.0, 2.0, OP.mult, OP.add)

        A_i = sc("A_i"); B_i = sc("B_i"); A_t = sc("A_t"); B_t = sc("B_t")
        t_a = sc("t_a"); t_b = sc("t_b")
        # A_i = e0*si0 + e1 + e2*0.3*img_auth
        nc.vector.scalar_tensor_tensor(t_a, img_auth, 0.3, e2, OP.mult, OP.mult)
        nc.vector.tensor_mul(t_b, si0, e0)
        nc.vector.tensor_add(t_a, t_a, t_b)
        nc.vector.tensor_add(A_i, t_a, e1)
        # B_i = e2*(1-img_auth)*img_imp
        nc.vector.tensor_scalar(t_a, img_auth, -1.0, 1.0, OP.mult, OP.add)
        nc.vector.tensor_mul(t_a, t_a, img_imp)
        nc.vector.tensor_mul(B_i, t_a, e2)
        # A_t = e0*st0 + e1*0.3*text_auth + e2
        nc.vector.scalar_tensor_tensor(t_a, text_auth, 0.3, e1, OP.mult, OP.mult)
        nc.vector.tensor_mul(t_b, st0, e0)
        nc.vector.tensor_add(t_a, t_a, t_b)
        nc.vector.tensor_add(A_t, t_a, e2)
        # B_t = e1*(1-text_auth)*text_imp
        nc.vector.tensor_scalar(t_a, text_auth, -1.0, 1.0, OP.mult, OP.add)
        nc.vector.tensor_mul(t_a, t_a, text_imp)
        nc.vector.tensor_mul(B_t, t_a, e1)

        # ---------------- tiny gate MLPs (transposed space) ----------------
        # Emitted before z1(0) on PE; the chain finishes while tile-0's
        # combine/XBAR are still in flight, so it doesn't gate z1(0).
        hq = singles.tile([1, B_CORE], bf16, tag="hq")   # q_att = .5 + .5*hq
        hw = singles.tile([1, B_CORE], bf16, tag="hw")   # img_w = .5 + .5*hw
        for n in range(N_TILES):
            sl = slice(n * TILE_N, (n + 1) * TILE_N)
            ps1 = ps.tile([64, TILE_N], f32, tag="ps", name="ps1")
            nc.tensor.matmul(ps1, qaw1, qualT[:, sl], start=True, stop=True)
            g1q = smallp.tile([64, TILE_N], bf16, tag="qg1", name="g1q")
            nc.scalar.activation(g1q, ps1, AF.Gelu, bias=qab1)
            ps2 = ps.tile([32, TILE_N], f32, tag="ps", name="ps2")
            nc.tensor.matmul(ps2, qaw2, g1q, start=True, stop=True)
            g2q = smallp.tile([32, TILE_N], bf16, tag="qg2", name="g2q")
            nc.scalar.activation(g2q, ps2, AF.Gelu, bias=qab2)
            ps3 = ps.tile([1, TILE_N], f32, tag="ps", name="ps3")
            nc.tensor.matmul(ps3, qaw3, g2q, start=True, stop=True)
            nc.scalar.activation(hq[:, sl], ps3, AF.Tanh, bias=qab3h, scale=0.5)

            psm1 = ps.tile([32, TILE_N], f32, tag="ps", name="psm1")
            nc.tensor.matmul(psm1, miw1p, qualT[:, sl], start=True, stop=True)
            mg = smallp.tile([32, TILE_N], bf16, tag="mg", name="mg")
            nc.scalar.activation(mg, psm1, AF.Gelu, bias=mib1)
            psm2 = ps.tile([1, TILE_N], f32, tag="ps", name="psm2")
            nc.tensor.matmul(psm2, wdiff, mg, start=True, stop=True)
            nc.scalar.activation(hw[:, sl], psm2, AF.Tanh, bias=db, scale=0.5)

        # difficulty rows: dhb = d/2, e = 1 + d/2  (comp = (e + dhb*st)*fin)
        dhb_row = singles.tile([1, B_CORE], bf16, tag="dhb_row")
        nc.vector.tensor_scalar(dhb_row, dT_row, 0.5, None, OP.mult)
        e_row = singles.tile([1, B_CORE], bf16, tag="e_row")
        nc.vector.tensor_scalar(e_row, dT_row, 0.5, 1.0, OP.mult, OP.add)
        ones_row = singles.tile([1, 128], bf16, tag="ones_row")
        nc.vector.memset(ones_row, 1.0)

        # ---------------- main-loop emit helpers ----------------
        fin_specs = [(0, 2, A_i, B_i), (1, 3, A_t, B_t)]

        def emit_combine(t, in_sb):
            """fin_rm[pi] = [128, 4(c), 512(f)] bf16, rows row-major."""
            fin_rm = []
            for pi, (bfi, efi, Ac, Bc) in enumerate(fin_specs):
                ft = finp.tile([128, PC, H], bf16, tag="fin", name="ft")
                for c in range(PC):
                    g = t * PC + c
                    tmp = tmpp.tile([128, H], bf16, tag="ctmp", name="tmp")
                    nc.vector.tensor_scalar(tmp, in_sb[efi][:, c, :],
                                            Bc[:, g:g + 1], None, OP.mult)
                    nc.vector.scalar_tensor_tensor(ft[:, c, :], in_sb[bfi][:, c, :],
                                                   Ac[:, g:g + 1], tmp,
                                                   OP.mult, OP.add)
                fin_rm.append(ft)
            return fin_rm

        def emit_transpose(fin_rm):
            """finT_blk[pi]: [128, c, fc, 128]; block (c,fc) = fin[c*128+q, fc*128+p].

            XBAR semantics with out [128, 16, 128] and SBUF in [128, 2048]:
            out[p, j, q] = in[q, j*128+p], i.e. j = c*4+fc."""
            finT = []
            for pi in range(2):
                fb = fintp.tile([128, PC, PC, 128], bf16, tag="finT", name="fb")
                if use_xbar:
                    nc.sync.dma_start(
                        out=fb, in_=fin_rm[pi].rearrange("p c f -> p (c f)"),
                        transpose=True)
                else:
                    for c in range(PC):
                        for fc in range(PC):
                            pst = ps.tile([128, 128], bf16, tag="ps", name="pst")
                            nc.tensor.transpose(
                                pst, fin_rm[pi][:, c, fc * 128:(fc + 1) * 128], ident)
                            if (c * PC + fc) % 2 == 0:
                                nc.vector.tensor_copy(fb[:, c, fc, :], pst)
                            else:
                                nc.scalar.activation(fb[:, c, fc, :], pst, AF.Copy)
                finT.append(fb)
            return finT

        def finT_view(fb, fc):
            # [128, 4(c), 128(q)] strided view = feature chunk fc, rows linear
            return fb[:, :, fc, :]

        def emit_f8cast(finT):
            """bf16 finT -> fp8 [128, 4(k), 512(rows linear)], DVE/ACT split."""
            f8T = []
            for pi in range(2):
                f8 = f8p.tile([128, PC, TILE_N], fp8, tag="f8", name="f8")
                for k in range(PC):
                    if (pi + k) % 2 == 0:
                        nc.vector.tensor_copy(f8[:, k, :], finT_view(finT[pi], k))
                    else:
                        nc.scalar.activation(f8[:, k, :], finT_view(finT[pi], k),
                                             AF.Copy)
                f8T.append(f8)
            return f8T

        def emit_z1_g1(t, finT, f8T):
            tsl = slice(t * TILE_N, (t + 1) * TILE_N)
            g1 = [g1p.tile([128, PC, TILE_N], fp8 if use_fp8 else bf16,
                           tag="g1", name="g1") for _ in range(2)]
            g1_scale = 1.0 / W2_SCALE if use_fp8 else 1.0
            for m in range(PC):
                ms = slice(m * 128, (m + 1) * 128)
                zp = [ps.tile([128, TILE_N], f32, tag="ps", name="z1") for _ in range(2)]
                if use_fp8:
                    for kp in range(2):
                        for pi in range(2):
                            nc.tensor.matmul(zp[pi], dcw1[:, 2 * kp:2 * kp + 2, ms],
                                             f8T[pi][:, 2 * kp:2 * kp + 2, :],
                                             perf_mode=PM.DoubleRow,
                                             start=(kp == 0), stop=False)
                else:
                    for k in range(PC):
                        for pi in range(2):
                            nc.tensor.matmul(zp[pi], dcw1[:, k, ms],
                                             finT_view(finT[pi], k),
                                             start=(k == 0), stop=False)
                for pi in range(2):
                    nc.tensor.matmul(zp[pi], dcw1_lastp[:, ms], qualT[:, tsl],
                                     start=False, stop=True)
                    nc.scalar.activation(g1[pi][:, m, :], zp[pi], AF.Gelu,
                                         bias=dcb1[:, m:m + 1], scale=g1_scale)
            return g1

        def rep0(ap2d, n):
            """[128, F] AP -> [128, n(stride 0), F] broadcast view."""
            return dataclasses.replace(ap2d, ap=[ap2d.ap[0], [0, n], *ap2d.ap[1:]])

        def emit_z2_st_pi(t, g1, pi):
            """z2 + tanh for one feature pair; returns st tile [128, 4, 512]."""
            stt = stp.tile([128, PC, TILE_N], bf16, tag="st", name="st")
            for fc in range(PC):
                ms = slice(fc * 128, (fc + 1) * 128)
                zp = ps.tile([128, TILE_N], f32, tag="ps", name="z2")
                if use_fp8:
                    for kp in range(2):
                        nc.tensor.matmul(zp, dcw2x[:, 2 * kp:2 * kp + 2, ms],
                                         g1[pi][:, 2 * kp:2 * kp + 2, :],
                                         perf_mode=PM.DoubleRow,
                                         start=(kp == 0), stop=(kp == 1))
                    st_scale = 0.5 / W2_SCALE
                else:
                    for k in range(PC):
                        nc.tensor.matmul(zp, dcw2x[:, k, ms], g1[pi][:, k, :],
                                         start=(k == 0), stop=(k == PC - 1))
                    st_scale = 0.5
                nc.scalar.activation(stt[:, fc, :], zp, AF.Tanh,
                                     bias=dcb2h[:, fc:fc + 1], scale=st_scale)
            return stt

        def emit_comp_pi(t, stt, finT, pi):
            tsl = slice(t * TILE_N, (t + 1) * TILE_N)
            # comp = (E + Dh*st) * fin, [128, 4(fc), 512] wide ops
            u = uvp.tile([128, PC, TILE_N], bf16, tag="u", name="u")
            nc.vector.tensor_mul(u, stt, rep0(Dhb[:, tsl], PC))
            n2 = uvp.tile([128, PC, TILE_N], bf16, tag="u", name="n2")
            nc.vector.tensor_add(n2, u, rep0(Eb[:, tsl], PC))
            cpt = compp.tile([128, PC, TILE_N], bf16, tag="comp", name="cpt")
            nc.vector.tensor_mul(
                cpt.rearrange("p f (c q) -> p f c q", q=128),
                n2.rearrange("p f (c q) -> p f c q", q=128),
                finT[pi].rearrange("p c f q -> p f c q"))
            return cpt

        def emit_attention_pi(t, cpt, pi, outs):
            ocol = 1 - pi
            for r in range(PC):
                ap = ps.tile([128, H], f32, tag="ps", name="att")
                for k in range(PC):
                    nc.tensor.matmul(ap, cpt[:, k, r * 128:(r + 1) * 128],
                                     wc[:, k, :], start=(k == 0),
                                     stop=(not use_bvo and k == PC - 1))
                if use_bvo:
                    nc.tensor.matmul(ap, ones_row, bvo_sb, start=False, stop=True)
                # gate (w*q_att) applied per-row via the copy's scale
                g = t * PC + r
                wq_col = wq_rm[:, g:g + 1, pi:pi + 1].rearrange("p a b -> p (a b)")
                if (r + pi) % 2 == 0:
                    nc.scalar.activation(outs[ocol][:, r, :], ap, AF.Copy,
                                         scale=wq_col)
                else:
                    nc.vector.tensor_scalar(outs[ocol][:, r, :], ap,
                                            wq_col, None, OP.mult)

        def emit_out_dma(t, outs):
            if t == N_TILES - 1:
                # tail: per-chunk stores on the (now idle) SP queue so the
                # last copies ship immediately instead of one big late DMA
                for ocol in range(2):
                    for r in range(PC):
                        rs = slice(t * TILE_N + r * 128, t * TILE_N + (r + 1) * 128)
                        nc.sync.dma_start(
                            out=d_out[rs, ocol * H:(ocol + 1) * H],
                            in_=outs[ocol][:, r, :])
                return
            for ocol in range(2):
                # gpsimd SWDGE: keeps late stores off the SP HWDGE ring
                nc.gpsimd.dma_start(
                    out=d_out[t * TILE_N:(t + 1) * TILE_N,
                              ocol * H:(ocol + 1) * H].rearrange(
                        "(r p) f -> p r f", p=128),
                    in_=outs[ocol])

        # ---------------- tile 0 head ----------------
        fin_rm = emit_combine(0, in_sb)
        finT = emit_transpose(fin_rm)
        f8T = emit_f8cast(finT) if use_fp8 else None
        in_sb2 = emit_loads(1)
        g1 = emit_z1_g1(0, finT, f8T)

        # ---------------- PE fill work (needed from att(0)/comp(0) on) -------
        # broadcast Dh/E rows to all partitions via K=1 PE matmuls
        Dhb = singles.tile([128, B_CORE], bf16, tag="Dhb")
        Eb = singles.tile([128, B_CORE], bf16, tag="Eb")
        for bi, (row, dst) in enumerate([(dhb_row, Dhb), (e_row, Eb)]):
            for c4 in range(N_TILES):
                sl = slice(c4 * TILE_N, (c4 + 1) * TILE_N)
                psb = ps.tile([128, TILE_N], f32, tag="ps", name="psb")
                nc.tensor.matmul(psb, ones_row, row[:, sl], start=True, stop=True)
                if (bi + c4) % 2 == 0:
                    nc.scalar.activation(dst[:, sl], psb, AF.Copy)
                else:
                    nc.vector.tensor_copy(dst[:, sl], psb)

        # row-major gates wq_rm[:,:,0]=w_t*q (pi=0: img comp -> text out),
        # wq_rm[:,:,1]=w_i*q, via tiny PE transposes of the hq/hw rows.
        hq_rm = singles.tile([128, RC_TOT, 2], bf16, tag="hq_rm")
        for c in range(RC_TOT):
            cs = slice(c * 128, (c + 1) * 128)
            for col, row in ((0, hq), (1, hw)):
                pst = ps.tile([128, 1], bf16, tag="ps", name="pst")
                nc.tensor.transpose(pst, row[:, cs], ones_row[0:1, 0:1])
                nc.vector.tensor_copy(hq_rm[:, c, col:col + 1], pst)
        wq_rm = singles.tile([128, RC_TOT, 2], f32, tag="wq_rm")
        gtmp = singles.tile([128, RC_TOT], f32, tag="gtmp")
        hqc = hq_rm[:, :, 0:1].rearrange("p c 1 -> p c")
        hwc = hq_rm[:, :, 1:2].rearrange("p c 1 -> p c")
        # wq_t = .25(1+hq)(1-hw); wq_i = .25(1+hq)(1+hw)
        nc.vector.tensor_scalar(gtmp, hwc, -0.25, 0.25, OP.mult, OP.add)
        nc.vector.scalar_tensor_tensor(
            wq_rm[:, :, 0:1].rearrange("p c 1 -> p c"), hqc, 1.0, gtmp,
            OP.add, OP.mult)
        nc.vector.tensor_scalar(gtmp, hwc, 0.25, 0.25, OP.mult, OP.add)
        nc.vector.scalar_tensor_tensor(
            wq_rm[:, :, 1:2].rearrange("p c 1 -> p c"), hqc, 1.0, gtmp,
            OP.add, OP.mult)

        # W_c = wv @ wo
        wc = singles.tile([128, PC, H], bf16, tag="wc")
        for ic in range(PC):
            psw = ps.tile([128, H], f32, tag="ps", name="psw")
            for kc in range(PC):
                nc.tensor.matmul(psw, wvT[:, kc, ic * 128:(ic + 1) * 128],
                                 wo_sb[:, kc, :],
                                 start=(kc == 0), stop=(kc == PC - 1))
            nc.scalar.activation(wc[:, ic, :], psw, AF.Copy)
        trans_ctx.close()

        # ---------------- main loop ----------------
        # PE queue: ... z2(t,p0), z2(t,p1), z1(t+1), att(t,p0), att(t,p1) ...
        for t in range(N_TILES):
            if t + 1 < N_TILES:
                fin2 = emit_combine(t + 1, in_sb2)
            st0_t = emit_z2_st_pi(t, g1, 0)
            st1_t = emit_z2_st_pi(t, g1, 1)
            if t + 1 < N_TILES:
                finT2 = emit_transpose(fin2)
                f8T2 = emit_f8cast(finT2) if use_fp8 else None
                if t + 2 < N_TILES:
                    in_sb2 = emit_loads(t + 2)
                g1 = emit_z1_g1(t + 1, finT2, f8T2)
            else:
                finT2 = None
            outs = [outp.tile([128, PC, H], bf16, tag="out", name="ot")
                    for _ in range(2)]
            cpt0 = emit_comp_pi(t, st0_t, finT, 0)
            emit_attention_pi(t, cpt0, 0, outs)
            cpt1 = emit_comp_pi(t, st1_t, finT, 1)
            emit_attention_pi(t, cpt1, 1, outs)
            emit_out_dma(t, outs)
            finT = finT2

    nc.compile()
    _dedupe_ldweights(nc, mybir)
    return nc


def _dedupe_ldweights(nc, mybir):
    """Drop InstLdweights that reload the exact weights already resident in
    the PE array (no intervening loads). Only sync-free LDWs are removed."""
    removed = 0
    for blk in nc.m.functions[0].blocks:
        insts = list(blk.instructions)
        keep = []
        cur = None
        for i in insts:
            if getattr(i, 'engine', None) != mybir.EngineType.PE:
                keep.append(i)
                continue
            t = type(i).__name__
            if t == 'InstLdweights':
                ap = i.ins[0]
                key = (str(ap.memref), ap.offset, str(ap.ap), str(ap.dtype),
                       bool(getattr(i, 'is_transpose', False)),
                       str(getattr(i, 'perf_mode', None)),
                       str(getattr(i, 'tile_position', None)))
                si = i.sync_info
                has_sync = bool(si and (si.on_wait or si.on_update))
                if key == cur and not has_sync:
                    removed += 1
                    continue
                cur = key
                keep.append(i)
            elif t == 'InstMatmult':
                keep.append(i)
            else:
                cur = None
                keep.append(i)
        if removed:
            blk.instructions = keep
    return removed


def _get_program(use_bvo, use_xbar, use_fp8):
    key = ("nc", use_bvo, use_xbar, use_fp8)
    if key not in _CACHE:
        _CACHE[key] = _build_program(use_bvo, use_xbar, use_fp8)
    return _CACHE[key]


def kernel(**inputs) -> np.ndarray:
    global last_exec_time_ns, last_trace_path, last_scope_times
    import ml_dtypes
    from concourse.bass_utils import run_bass_kernel_spmd

    bf = ml_dtypes.bfloat16
    f8 = ml_dtypes.float8_e4m3

    use_xbar = os.environ.get("KERNEL_XBAR", "1") == "1"
    use_fp8 = os.environ.get("KERNEL_FP8", "1") == "1"
    use_bvo = bool(np.any(np.asarray(inputs["bv"])) or
                   np.any(np.asarray(inputs["bo"])))
    nc = _get_program(use_bvo, use_xbar, use_fp8)

    f = {k: np.ascontiguousarray(np.asarray(v, dtype=np.float32))
         for k, v in inputs.items()}
    missing_f = np.ascontiguousarray(
        np.asarray(inputs["missing_type"]).astype(np.float32))

    # host-staged weight transforms (layout/dtype only, plus the exact
    # bias collapse bvo = bv@wo + bo)
    wblob = np.zeros((64, 132), np.float32)
    wblob[0:11, 0:64] = f["qa_w1"]
    wblob[0:64, 64:96] = f["qa_w2"]
    wblob[0:32, 96:97] = f["qa_w3"]
    wblob[6:10, 97:129] = f["mi_w1"]       # padded mi input gather
    wblob[0:32, 131:132] = (f["mi_w2"][:, 0] - f["mi_w2"][:, 1])[:, None]
    bblob = np.zeros((64, 5), np.float32)
    bblob[0:64, 0] = f["qa_b1"]
    bblob[0:32, 1] = f["qa_b2"]
    bblob[0, 2] = f["qa_b3"][0] * 0.5
    bblob[0:32, 3] = f["mi_b1"]
    bblob[0, 4] = (f["mi_b2"][0] - f["mi_b2"][1]) * 0.5
    dcb = np.concatenate([f["dc_b1"].reshape(PC, 128).T,
                          (f["dc_b2"] * 0.5).reshape(PC, 128).T], axis=1)
    dcw1_lastp = np.zeros((11, H), np.float32)
    dcw1_lastp[10] = f["dc_w1"][H]
    bvo = (f["bv"].astype(np.float64) @ f["wo"].astype(np.float64)
           + f["bo"]).astype(np.float32)
    w2x = np.clip(f["dc_w2"] * W2_SCALE, -240.0, 240.0)

    weights = {
        "wblob": wblob.astype(bf),
        "bblob": np.ascontiguousarray(bblob),
        "dcb": np.ascontiguousarray(dcb),
        "dcw1": (np.clip(f["dc_w1"][:H] * W2_SCALE, -240, 240).astype(f8)
                 if use_fp8 else np.ascontiguousarray(f["dc_w1"][:H]).astype(bf)),
        "dcw1_lastp": ((dcw1_lastp * W2_SCALE).astype(bf)
                       if use_fp8 else dcw1_lastp.astype(bf)),
        "dcw2x": w2x.astype(f8) if use_fp8 else f["dc_w2"].astype(bf),
        "wv": f["wv"].astype(bf), "wo": f["wo"].astype(bf), "bvo": bvo,
    }

    feats_bf = {k: f[k].astype(bf) for k in
                ["image_feat", "text_feat", "enhanced_image_feat",
                 "enhanced_text_feat"]}
    qualT_bf = np.ascontiguousarray(f["quality"].T).astype(bf)

    in_maps = []
    for c in range(N_CORES):
        sl = slice(c * B_CORE, (c + 1) * B_CORE)
        m = {k: np.ascontiguousarray(v[sl]) for k, v in feats_bf.items()}
        m["quality"] = f["quality"][sl]
        m["qualT"] = np.ascontiguousarray(qualT_bf[:, sl])
        m["missing_f"] = missing_f[sl]
        m.update(weights)
        in_maps.append(m)

    trace = os.environ.get("KERNEL_TRACE", "0") == "1"
    res = run_bass_kernel_spmd(nc, in_maps, core_ids=list(range(N_CORES)),
                               trace=trace)
    last_exec_time_ns = res.exec_time_ns
    last_scope_times = res.per_core_scope_times
    if res.instructions_and_trace is not None:
        last_trace_path = res.instructions_and_trace[1]

    out = np.empty((B_FULL, 2 * H), dtype=np.float32)
    for c in range(N_CORES):
        out[c * B_CORE:(c + 1) * B_CORE] = res.results[c]["out"].astype(np.float32)
    return out
